# revision 20
# baseline (speedup 1.0000x reference)
"""Sharded kNN (ArgDistanceMeasure) on 8 TRN2 NeuronCores.

Strategy (FAISS-style sharded kNN), v3:
  - b (the database, [65536, 512]) is sharded row-wise across 8 cores
    (8192 rows each); a (queries, [2048, 512]) is replicated.
  - Ranking identity: argmin_j ||a_i - b_j + eps||^2 over j only needs the
    column-dependent part  score[i,j] = 2*a_i.b_j - c_j  (maximized), where
    c_j = ||b_j||^2 - 2*eps*sum(b_j).  Row-constant terms don't affect
    per-row ranking.
  - Columns of each 2048-wide chunk are host-permuted so that device
    position q holds the column with c-sorted rank (q % 1024)*2 + (q // 1024):
    one pairwise-max over the chunk halves groups PAIRS of c-adjacent columns
    (spread <= 2 ranks), so the c_j bias can be applied on the host to the
    1024 pair-maxima — the PE runs a pure GEMM.
  - Per [128 queries x 2048 cols] chunk:
      PE:  fp8-e4m3 DoubleRow GEMM (K=256/matmul) accumulating 2*cross into
           PSUM; 8 matmuls of N=512 (~1.73us/chunk warm).
      ACT: two 512-col copies PSUM[0:1024] -> SBUF fp16 (s16a).
      DVE: two 512-col tensor_max ops, each reading one s16a half (SBUF
           fp16) and one PSUM fp32 quadrant directly (dual-PSUM-operand TT
           is rejected by walrus; one PSUM operand is fine) -> m2 fp16
           pair-maxima.  Each PSUM quadrant has exactly ONE consumer
           (ACT-a, ACT-b, L1a, L1b), so every next-next-chunk matmul
           carries a single semaphore wait, and the quadrants free
           in a staggered pipeline well before the PE needs them.
      DMA: m2 [128, 1024] fp16 (256KB/chunk) -> DRAM out_q.
  - Startup: the first m-slice of a (queries 0-511) and the first half of
    b-chunk 0 are small dedicated DMAs so the first matmul issues ~4us
    earlier than a monolithic preload; remaining loads are dependency-gated
    behind the critical wave.  Dummy matmuls on memset scratch warm the PE
    HAM clock (cold 1.2GHz -> warm 2.4GHz) while the first DMAs land.
  - Host: subtracts the fp32 per-pair bias from the 1024 pair-maxima per
    chunk, keeps the top-32 pairs per (query, chunk), expands each to its 2
    c-adjacent columns (2048 candidates/query), recomputes the exact fp32
    reference distance and picks the final top-n with the reference's
    tie-break and buggy index bookkeeping.  (Validated in simulation:
    0/32768 mismatches.)
"""
import numpy as np

NA, D, NB = 2048, 512, 65536
NCORES = 8
NB_SHARD = NB // NCORES  # 8192
CHUNK = 2048             # chunk width (4 PSUM banks)
PAIR = CHUNK // 2        # 1024 pair-maxima per chunk
NSEL = 32                # pairs kept per (query, chunk) on the host
NDUMMY = 26              # PE warmup matmuls (N=128) on scratch: span the
                         # first-DMA wait AND the ~3.4us HAM activity window,
                         # so real matmuls start warm (2.4GHz) as soon as
                         # their data lands; N=128 keeps the leftover-dummy
                         # cost small once data is ready
EPS = 1e-6
M0 = 4                   # m-tiles covered by the first a m-slice


def build_kernel(na=NA, nb_shard=NB_SHARD, chunk=CHUNK):
    import concourse.mybir as mybir
    from concourse import bacc
    from concourse.tile import TileContext, add_dep_helper

    FP8 = mybir.dt.float8e4
    F16 = mybir.dt.float16
    F32 = mybir.dt.float32
    DR = mybir.MatmulPerfMode.DoubleRow

    nseg = nb_shard // chunk
    kt = D // 128            # 4 K-tiles of 128
    kp_n = kt // 2           # 2 DoubleRow pairs (K=256 each)
    mt = na // 128

    nc = bacc.Bacc()

    # DoubleRow operands are [128, 2, cols] (two K-slices packed per
    # partition).  a is split into a first m-slice (queries 0..511) and the
    # rest; b chunk 0 is split into halves so the first matmul's operands are
    # small, early DMAs.
    atm0_p = [
        nc.declare_dram_parameter(f"atm0p{kp}", [128, 2 * 128 * M0], FP8, isOutput=False)
        for kp in range(kp_n)
    ]
    atr_p = [
        nc.declare_dram_parameter(
            f"atrp{kp}", [128, 2 * 128 * (mt - M0)], FP8, isOutput=False
        )
        for kp in range(kp_n)
    ]
    bt0h_p = [
        [
            nc.declare_dram_parameter(
                f"bt0p{kp}{h}", [128, 2 * (chunk // 2)], FP8, isOutput=False
            )
            for h in ("a", "b")
        ]
        for kp in range(kp_n)
    ]
    bt_p = {
        (g, kp): nc.declare_dram_parameter(
            f"bt{g}p{kp}", [128, 2 * chunk], FP8, isOutput=False
        )
        for g in range(1, nseg)
        for kp in range(kp_n)
    }
    out_q = nc.declare_dram_parameter("out_q", [na, nseg * PAIR], F16, isOutput=True)

    with TileContext(nc) as tc:
        with (
            tc.tile_pool(name="weights", bufs=1) as wpool,
            tc.tile_pool(name="psum", bufs=2, space="PSUM") as ppool,
            tc.tile_pool(name="scores", bufs=4) as spool,
            tc.tile_pool(name="pairs", bufs=8) as mpool,
        ):
            # --- PE warmup: dummy DoubleRow matmuls on memset scratch ---
            wscr = wpool.tile([128, 2 * 128], FP8, tag="wscr", name="wscr")
            nc.gpsimd.memset(wscr, 0)
            w3 = wscr.rearrange("p (i c) -> p i c", i=2)
            # Per-j PSUM tiles (4 tags x bufs=2 x 1 bank = all 8 banks): each
            # consumer (ACT-a<-j0, ACT-b<-j1, L1a<-j2, L1b<-j3) then depends
            # on just its own quadrant's two matmuls, so copies start ~1us
            # before the chunk's last matmul and the PE never waits on the
            # (tile-granular) PSUM WAR chain.
            ps_first = ppool.tile([128, 512], F32, tag="score0", name="ps00")
            for _ in range(NDUMMY):
                nc.tensor.matmul(
                    ps_first[:, :128], w3, w3,
                    start=True, stop=True, perf_mode=DR,
                )

            # --- critical first wave: a m-slice 0, b chunk 0 halves.
            # The chunk-0 halves are separate CONTIGUOUS tiles: a DMA into a
            # strided [128,2,1024] view of one big tile generates 1KB packets
            # and runs ~4x slower than these 2KB-per-partition lines.
            atm0 = []
            bt0h = [[None, None] for _ in range(kp_n)]
            half = chunk // 2
            for kp in range(kp_n):
                for hh in range(2):
                    bt0h[kp][hh] = wpool.tile(
                        [128, 2 * half], FP8, tag=f"bt0p{kp}h{hh}", name=f"bt0p{kp}h{hh}"
                    )
            crit0 = nc.sync.dma_start(out=bt0h[0][0], in_=bt0h_p[0][0][:, :])
            for kp in range(kp_n):
                t = wpool.tile(
                    [128, 2 * 128 * M0], FP8, tag=f"atm0p{kp}", name=f"atm0p{kp}"
                )
                nc.sync.dma_start(out=t, in_=atm0_p[kp][:, :])
                atm0.append(t)
            nc.sync.dma_start(out=bt0h[1][0], in_=bt0h_p[1][0][:, :])
            for kp in range(kp_n):
                crit_dma = nc.sync.dma_start(out=bt0h[kp][1], in_=bt0h_p[kp][1][:, :])
            bt0h3 = [
                [t.rearrange("p (i c) -> p i c", i=2) for t in row] for row in bt0h
            ]
            # --- gated preloads: rest of a, b chunks 1..3 ---
            atr = []
            for kp in range(kp_n):
                t = wpool.tile(
                    [128, 2 * 128 * (mt - M0)], FP8, tag=f"atrp{kp}", name=f"atrp{kp}"
                )
                d = nc.sync.dma_start(out=t, in_=atr_p[kp][:, :])
                add_dep_helper(d.ins, crit_dma.ins, True, "preload priority")
                atr.append(t)
            bt_t = {}
            for g in range(1, nseg):
                for kp in range(kp_n):
                    t = wpool.tile(
                        [128, 2 * chunk], FP8, tag=f"bt{g}p{kp}", name=f"bt{g}p{kp}"
                    )
                    d = nc.sync.dma_start(out=t, in_=bt_p[(g, kp)][:, :])
                    add_dep_helper(d.ins, crit_dma.ins, True, "preload priority")
                    bt_t[(g, kp)] = t

            atm0_3 = [t.rearrange("p (i c) -> p i c", i=2) for t in atm0]
            atr_3 = [t.rearrange("p (i c) -> p i c", i=2) for t in atr]
            bt3 = {
                (g, kp): bt_t[(g, kp)].rearrange("p (i c) -> p i c", i=2)
                for g in range(1, nseg)
                for kp in range(kp_n)
            }

            for s in range(nseg):
                for m in range(mt):
                    ps_j = []
                    for j in range(4):
                        if s == 0 and m == 0 and j == 0:
                            ps_j.append(ps_first)
                        else:
                            ps_j.append(
                                ppool.tile(
                                    [128, 512], F32, tag=f"score{j}", name=f"ps{j}"
                                )
                            )
                    # j grouped in halves so chunk 0's matmuls chase the
                    # half-DMAs; kp inner per half for PSUM accumulate.
                    for h in range(2):
                        for kp in range(kp_n):
                            if m < M0:
                                lhsT3 = atm0_3[kp][:, :, m * 128 : (m + 1) * 128]
                            else:
                                lhsT3 = atr_3[kp][:, :, (m - M0) * 128 : (m - M0 + 1) * 128]
                            for j in (2 * h, 2 * h + 1):
                                if s == 0:
                                    rhs3 = bt0h3[kp][h][:, :, (j % 2) * 512 : (j % 2 + 1) * 512]
                                else:
                                    rhs3 = bt3[(s, kp)][:, :, j * 512 : (j + 1) * 512]
                                nc.tensor.matmul(
                                    ps_j[j],
                                    lhsT3,
                                    rhs3,
                                    start=(kp == 0),
                                    stop=(kp == kp_n - 1),
                                    perf_mode=DR,
                                )
                    s16a = spool.tile([128, PAIR], F16, tag="s16a", name="s16a")
                    nc.scalar.copy(out=s16a[:, :512], in_=ps_j[0])
                    nc.scalar.copy(out=s16a[:, 512:], in_=ps_j[1])
                    m2 = mpool.tile([128, PAIR], F16, tag="m2", name="m2")
                    nc.vector.tensor_max(m2[:, :512], s16a[:, :512], ps_j[2])
                    nc.vector.tensor_max(m2[:, 512:], s16a[:, 512:], ps_j[3])
                    nc.sync.dma_start(
                        out=out_q[m * 128 : (m + 1) * 128, s * PAIR : (s + 1) * PAIR],
                        in_=m2,
                    )
    nc.compile()
    return nc


def make_in_maps(a, b):
    """Pack per-core inputs.  Columns of each 2048-wide chunk are permuted so
    that device position q holds the column with c-sorted rank
    (q % 1024)*2 + (q // 1024) — making the pairwise-max mates c-adjacent
    (spread <= 2 ranks), so the bias can be applied on the host to the 1024
    pair-maxima.  Returns (in_maps, ranks, cpair) where ranks[core][s][r] is
    the local column with the r-th smallest c and cpair[core][s][t] the mean
    c of pair t."""
    import ml_dtypes

    kt = D // 128
    kp_n = kt // 2
    aT8 = (2.0 * a).T.astype(ml_dtypes.float8_e4m3)   # [512, NA]
    bT8 = b.T.astype(ml_dtypes.float8_e4m3)           # [512, NB]
    b2 = np.sum(b * b, axis=1)
    sb = b.sum(axis=1)
    c = (b2 - np.float32(2.0 * EPS) * sb).astype(np.float32)
    nseg = NB_SHARD // CHUNK
    q = np.arange(CHUNK)
    r_of_q = (q % PAIR) * 2 + (q // PAIR)
    half = CHUNK // 2

    def pair_pack(mat, kp):
        # [128, 2*cols]: slot i holds K-tile (kp*2+i) rows of mat
        lo = mat[(kp * 2) * 128 : (kp * 2 + 1) * 128, :]
        hi = mat[(kp * 2 + 1) * 128 : (kp * 2 + 2) * 128, :]
        return np.ascontiguousarray(np.concatenate([lo, hi], axis=1))

    in_maps = []
    all_ranks = []
    all_cp = []
    for core in range(NCORES):
        sl = slice(core * NB_SHARD, (core + 1) * NB_SHARD)
        bT = bT8[:, sl]
        c_core = c[core * NB_SHARD : (core + 1) * NB_SHARD]
        ranks = []
        cpair = []
        im = {}
        for kp in range(kp_n):
            im[f"atm0p{kp}"] = pair_pack(aT8[:, : 128 * M0], kp)
            im[f"atrp{kp}"] = pair_pack(aT8[:, 128 * M0 :], kp)
        for g in range(nseg):
            cch = c_core[g * CHUNK : (g + 1) * CHUNK]
            rank = np.argsort(cch, kind="stable")
            ranks.append(rank)
            perm = rank[r_of_q]
            cols = bT[:, g * CHUNK : (g + 1) * CHUNK][:, perm]
            cpair.append(cch[rank.reshape(PAIR, 2)].mean(axis=1).astype(np.float32))
            for kp in range(kp_n):
                full = pair_pack(np.ascontiguousarray(cols), kp)  # [128, 2*CHUNK]
                if g == 0:
                    im[f"bt0p{kp}a"] = np.ascontiguousarray(
                        np.concatenate(
                            [full[:, 0:half], full[:, CHUNK : CHUNK + half]], axis=1
                        )
                    )
                    im[f"bt0p{kp}b"] = np.ascontiguousarray(
                        np.concatenate(
                            [full[:, half:CHUNK], full[:, CHUNK + half :]], axis=1
                        )
                    )
                else:
                    im[f"bt{g}p{kp}"] = full
        in_maps.append(im)
        all_ranks.append(ranks)
        all_cp.append(cpair)
    return in_maps, all_ranks, all_cp


def merge_results(a, b, n, b_batch_size, results, all_ranks, all_cp):
    """Subtract the fp32 pair bias from each chunk's 1024 pair-maxima, keep
    the top-NSEL pairs per (query, chunk), expand each to its 2 c-adjacent
    columns, refine with the exact fp32 reference distance, pick final top-n
    (ties -> lowest index), apply the reference's bookkeeping."""
    nseg = NB_SHARD // CHUNK
    na = a.shape[0]
    cand_parts = []
    for core in range(NCORES):
        oq = results[core]["out_q"].astype(np.float32)  # [NA, nseg*PAIR]
        for s in range(nseg):
            adj = oq[:, s * PAIR : (s + 1) * PAIR] - all_cp[core][s][None, :]
            top = np.argpartition(-adj, NSEL, axis=1)[:, :NSEL]
            rank = all_ranks[core][s]
            base = core * NB_SHARD + s * CHUNK
            for k in range(2):
                cand_parts.append(rank[2 * top + k] + base)
    cand = np.concatenate(cand_parts, axis=1)  # [NA, NCORES*nseg*NSEL*2]

    a2 = np.sum(a * a, axis=1)
    sa = np.sum(a, axis=1)
    b2 = np.sum(b * b, axis=1)
    sb = np.sum(b, axis=1)
    d = a.shape[1]
    out = np.empty((na, n), dtype=np.int64)
    CHQ = 128
    eps = np.float32(EPS)
    for q0 in range(0, na, CHQ):
        q1 = min(q0 + CHQ, na)
        Cc = cand[q0:q1]
        Bc = b[Cc]
        cross = np.matmul(Bc, a[q0:q1, :, None])[..., 0].astype(np.float32)
        sq = (
            a2[q0:q1, None]
            + b2[Cc]
            - np.float32(2.0) * cross
            + np.float32(2.0) * eps * (sa[q0:q1, None] - sb[Cc])
            + np.float32(d) * eps * eps
        )
        dist = np.sqrt(np.maximum(sq, np.float32(0.0)))
        ordr = np.lexsort((Cc, dist), axis=1)[:, :n]
        rows = np.arange(q1 - q0)[:, None]
        out[q0:q1] = Cc[rows, ordr]
    buggy = (out % b_batch_size) + (out // b_batch_size)
    return buggy.astype(np.int32)


def kernel(a, b, n, b_batch_size, trace=False):
    from concourse.bass_utils import run_bass_kernel_spmd

    a = np.ascontiguousarray(np.asarray(a, dtype=np.float32))
    b = np.ascontiguousarray(np.asarray(b, dtype=np.float32))
    n = int(n)
    b_batch_size = int(b_batch_size)

    nc = build_kernel()
    in_maps, all_ranks, all_cp = make_in_maps(a, b)
    res = run_bass_kernel_spmd(
        nc, in_maps, core_ids=list(range(NCORES)), trace=trace
    )
    out = merge_results(a, b, n, b_batch_size, res.results, all_ranks, all_cp)
    if trace:
        return out, res
    return out


# revision 23
# speedup vs baseline: 1.0053x; 1.0053x over previous
"""Sharded kNN (ArgDistanceMeasure) on 8 TRN2 NeuronCores.

Strategy (FAISS-style sharded kNN), v3:
  - b (the database, [65536, 512]) is sharded row-wise across 8 cores
    (8192 rows each); a (queries, [2048, 512]) is replicated.
  - Ranking identity: argmin_j ||a_i - b_j + eps||^2 over j only needs the
    column-dependent part  score[i,j] = 2*a_i.b_j - c_j  (maximized), where
    c_j = ||b_j||^2 - 2*eps*sum(b_j).  Row-constant terms don't affect
    per-row ranking.
  - Columns of each 2048-wide chunk are host-permuted so that device
    position q holds the column with c-sorted rank (q % 1024)*2 + (q // 1024):
    one pairwise-max over the chunk halves groups PAIRS of c-adjacent columns
    (spread <= 2 ranks), so the c_j bias can be applied on the host to the
    1024 pair-maxima — the PE runs a pure GEMM.
  - Per [128 queries x 2048 cols] chunk:
      PE:  fp8-e4m3 DoubleRow GEMM (K=256/matmul) accumulating 2*cross into
           PSUM; 8 matmuls of N=512 (~1.73us/chunk warm).
      ACT: two 512-col copies PSUM[0:1024] -> SBUF fp16 (s16a).
      DVE: two 512-col tensor_max ops, each reading one s16a half (SBUF
           fp16) and one PSUM fp32 quadrant directly (dual-PSUM-operand TT
           is rejected by walrus; one PSUM operand is fine) -> m2 fp16
           pair-maxima.  Each PSUM quadrant has exactly ONE consumer
           (ACT-a, ACT-b, L1a, L1b), so every next-next-chunk matmul
           carries a single semaphore wait, and the quadrants free
           in a staggered pipeline well before the PE needs them.
      DMA: m2 [128, 1024] fp16 (256KB/chunk) -> DRAM out_q.
  - Startup: the first m-slice of a (queries 0-511) and the first half of
    b-chunk 0 are small dedicated DMAs so the first matmul issues ~4us
    earlier than a monolithic preload; remaining loads are dependency-gated
    behind the critical wave.  Dummy matmuls on memset scratch warm the PE
    HAM clock (cold 1.2GHz -> warm 2.4GHz) while the first DMAs land.
  - Host: subtracts the fp32 per-pair bias from the 1024 pair-maxima per
    chunk, keeps the top-32 pairs per (query, chunk), expands each to its 2
    c-adjacent columns (2048 candidates/query), recomputes the exact fp32
    reference distance and picks the final top-n with the reference's
    tie-break and buggy index bookkeeping.  (Validated in simulation:
    0/32768 mismatches.)
"""
import numpy as np

NA, D, NB = 2048, 512, 65536
NCORES = 8
NB_SHARD = NB // NCORES  # 8192
CHUNK = 2048             # chunk width (4 PSUM banks)
PAIR = CHUNK // 2        # 1024 pair-maxima per chunk
NSEL = 32                # pairs kept per (query, chunk) on the host
NDUMMY = 7               # PE warmup matmuls (N=512) on scratch: span the
                         # first-DMA wait AND the ~3.4us HAM activity window,
                         # so real matmuls start warm (2.4GHz) as soon as
                         # their data lands
EPS = 1e-6
M0 = 4                   # m-tiles covered by the first a m-slice


def build_kernel(na=NA, nb_shard=NB_SHARD, chunk=CHUNK):
    import concourse.mybir as mybir
    from concourse import bacc
    from concourse.tile import TileContext, add_dep_helper

    FP8 = mybir.dt.float8e4
    F16 = mybir.dt.float16
    F32 = mybir.dt.float32
    DR = mybir.MatmulPerfMode.DoubleRow

    nseg = nb_shard // chunk
    kt = D // 128            # 4 K-tiles of 128
    kp_n = kt // 2           # 2 DoubleRow pairs (K=256 each)
    mt = na // 128

    nc = bacc.Bacc()

    # DoubleRow operands are [128, 2, cols] (two K-slices packed per
    # partition).  a is split into a first m-slice (queries 0..511) and the
    # rest; b chunk 0 is split into halves so the first matmul's operands are
    # small, early DMAs.
    atm0_p = [
        nc.declare_dram_parameter(f"atm0p{kp}", [128, 2 * 128 * M0], FP8, isOutput=False)
        for kp in range(kp_n)
    ]
    atr_p = [
        nc.declare_dram_parameter(
            f"atrp{kp}", [128, 2 * 128 * (mt - M0)], FP8, isOutput=False
        )
        for kp in range(kp_n)
    ]
    bt0h_p = [
        [
            nc.declare_dram_parameter(
                f"bt0p{kp}{h}", [128, 2 * (chunk // 2)], FP8, isOutput=False
            )
            for h in ("a", "b")
        ]
        for kp in range(kp_n)
    ]
    bt_p = {
        (g, kp): nc.declare_dram_parameter(
            f"bt{g}p{kp}", [128, 2 * chunk], FP8, isOutput=False
        )
        for g in range(1, nseg)
        for kp in range(kp_n)
    }
    out_q = nc.declare_dram_parameter("out_q", [na, nseg * PAIR], F16, isOutput=True)

    with TileContext(nc) as tc:
        with (
            tc.tile_pool(name="weights", bufs=1) as wpool,
            tc.tile_pool(name="psum", bufs=2, space="PSUM") as ppool,
            tc.tile_pool(name="scores", bufs=4) as spool,
            tc.tile_pool(name="pairs", bufs=8) as mpool,
        ):
            # --- PE warmup: dummy DoubleRow matmuls on memset scratch ---
            wscr = wpool.tile([128, 2 * 512], FP8, tag="wscr", name="wscr")
            nc.gpsimd.memset(wscr, 0)
            w3 = wscr.rearrange("p (i c) -> p i c", i=2)
            # Per-j PSUM tiles (4 tags x bufs=2 x 1 bank = all 8 banks): each
            # consumer (ACT-a<-j0, ACT-b<-j1, L1a<-j2, L1b<-j3) then depends
            # on just its own quadrant's two matmuls, so copies start ~1us
            # before the chunk's last matmul and the PE never waits on the
            # (tile-granular) PSUM WAR chain.
            ps_first = ppool.tile([128, 512], F32, tag="score0", name="ps00")
            for _ in range(NDUMMY):
                nc.tensor.matmul(
                    ps_first, w3[:, :, :128], w3,
                    start=True, stop=True, perf_mode=DR,
                )

            # --- critical first wave: a m-slice 0, b chunk 0 halves.
            # The chunk-0 halves are separate CONTIGUOUS tiles: a DMA into a
            # strided [128,2,1024] view of one big tile generates 1KB packets
            # and runs ~4x slower than these 2KB-per-partition lines.
            atm0 = []
            bt0h = [[None, None] for _ in range(kp_n)]
            half = chunk // 2
            for kp in range(kp_n):
                for hh in range(2):
                    bt0h[kp][hh] = wpool.tile(
                        [128, 2 * half], FP8, tag=f"bt0p{kp}h{hh}", name=f"bt0p{kp}h{hh}"
                    )
            crit0 = nc.sync.dma_start(out=bt0h[0][0], in_=bt0h_p[0][0][:, :])
            for kp in range(kp_n):
                t = wpool.tile(
                    [128, 2 * 128 * M0], FP8, tag=f"atm0p{kp}", name=f"atm0p{kp}"
                )
                nc.sync.dma_start(out=t, in_=atm0_p[kp][:, :])
                atm0.append(t)
            nc.sync.dma_start(out=bt0h[1][0], in_=bt0h_p[1][0][:, :])
            for kp in range(kp_n):
                crit_dma = nc.sync.dma_start(out=bt0h[kp][1], in_=bt0h_p[kp][1][:, :])
            bt0h3 = [
                [t.rearrange("p (i c) -> p i c", i=2) for t in row] for row in bt0h
            ]
            # --- gated preloads: rest of a, b chunks 1..3 ---
            atr = []
            for kp in range(kp_n):
                t = wpool.tile(
                    [128, 2 * 128 * (mt - M0)], FP8, tag=f"atrp{kp}", name=f"atrp{kp}"
                )
                d = nc.sync.dma_start(out=t, in_=atr_p[kp][:, :])
                add_dep_helper(d.ins, crit_dma.ins, True, "preload priority")
                atr.append(t)
            bt_t = {}
            for g in range(1, nseg):
                for kp in range(kp_n):
                    t = wpool.tile(
                        [128, 2 * chunk], FP8, tag=f"bt{g}p{kp}", name=f"bt{g}p{kp}"
                    )
                    d = nc.sync.dma_start(out=t, in_=bt_p[(g, kp)][:, :])
                    add_dep_helper(d.ins, crit_dma.ins, True, "preload priority")
                    bt_t[(g, kp)] = t

            atm0_3 = [t.rearrange("p (i c) -> p i c", i=2) for t in atm0]
            atr_3 = [t.rearrange("p (i c) -> p i c", i=2) for t in atr]
            bt3 = {
                (g, kp): bt_t[(g, kp)].rearrange("p (i c) -> p i c", i=2)
                for g in range(1, nseg)
                for kp in range(kp_n)
            }

            for s in range(nseg):
                for m in range(mt):
                    ps_j = []
                    for j in range(4):
                        if s == 0 and m == 0 and j == 0:
                            ps_j.append(ps_first)
                        else:
                            ps_j.append(
                                ppool.tile(
                                    [128, 512], F32, tag=f"score{j}", name=f"ps{j}"
                                )
                            )
                    # j grouped in halves so chunk 0's matmuls chase the
                    # half-DMAs; kp inner per half for PSUM accumulate.
                    for h in range(2):
                        for kp in range(kp_n):
                            if m < M0:
                                lhsT3 = atm0_3[kp][:, :, m * 128 : (m + 1) * 128]
                            else:
                                lhsT3 = atr_3[kp][:, :, (m - M0) * 128 : (m - M0 + 1) * 128]
                            for j in (2 * h, 2 * h + 1):
                                if s == 0:
                                    rhs3 = bt0h3[kp][h][:, :, (j % 2) * 512 : (j % 2 + 1) * 512]
                                else:
                                    rhs3 = bt3[(s, kp)][:, :, j * 512 : (j + 1) * 512]
                                nc.tensor.matmul(
                                    ps_j[j],
                                    lhsT3,
                                    rhs3,
                                    start=(kp == 0),
                                    stop=(kp == kp_n - 1),
                                    perf_mode=DR,
                                )
                    s16a = spool.tile([128, PAIR], F16, tag="s16a", name="s16a")
                    nc.scalar.copy(out=s16a[:, :512], in_=ps_j[0])
                    nc.scalar.copy(out=s16a[:, 512:], in_=ps_j[1])
                    m2 = mpool.tile([128, PAIR], F16, tag="m2", name="m2")
                    nc.vector.tensor_max(m2[:, :512], s16a[:, :512], ps_j[2])
                    nc.vector.tensor_max(m2[:, 512:], s16a[:, 512:], ps_j[3])
                    nc.sync.dma_start(
                        out=out_q[m * 128 : (m + 1) * 128, s * PAIR : (s + 1) * PAIR],
                        in_=m2,
                    )
    nc.compile()
    return nc


def make_in_maps(a, b):
    """Pack per-core inputs.  Columns of each 2048-wide chunk are permuted so
    that device position q holds the column with c-sorted rank
    (q % 1024)*2 + (q // 1024) — making the pairwise-max mates c-adjacent
    (spread <= 2 ranks), so the bias can be applied on the host to the 1024
    pair-maxima.  Returns (in_maps, ranks, cpair) where ranks[core][s][r] is
    the local column with the r-th smallest c and cpair[core][s][t] the mean
    c of pair t."""
    import ml_dtypes

    kt = D // 128
    kp_n = kt // 2
    aT8 = (2.0 * a).T.astype(ml_dtypes.float8_e4m3)   # [512, NA]
    bT8 = b.T.astype(ml_dtypes.float8_e4m3)           # [512, NB]
    b2 = np.sum(b * b, axis=1)
    sb = b.sum(axis=1)
    c = (b2 - np.float32(2.0 * EPS) * sb).astype(np.float32)
    nseg = NB_SHARD // CHUNK
    q = np.arange(CHUNK)
    r_of_q = (q % PAIR) * 2 + (q // PAIR)
    half = CHUNK // 2

    def pair_pack(mat, kp):
        # [128, 2*cols]: slot i holds K-tile (kp*2+i) rows of mat
        lo = mat[(kp * 2) * 128 : (kp * 2 + 1) * 128, :]
        hi = mat[(kp * 2 + 1) * 128 : (kp * 2 + 2) * 128, :]
        return np.ascontiguousarray(np.concatenate([lo, hi], axis=1))

    in_maps = []
    all_ranks = []
    all_cp = []
    for core in range(NCORES):
        sl = slice(core * NB_SHARD, (core + 1) * NB_SHARD)
        bT = bT8[:, sl]
        c_core = c[core * NB_SHARD : (core + 1) * NB_SHARD]
        ranks = []
        cpair = []
        im = {}
        for kp in range(kp_n):
            im[f"atm0p{kp}"] = pair_pack(aT8[:, : 128 * M0], kp)
            im[f"atrp{kp}"] = pair_pack(aT8[:, 128 * M0 :], kp)
        for g in range(nseg):
            cch = c_core[g * CHUNK : (g + 1) * CHUNK]
            rank = np.argsort(cch, kind="stable")
            ranks.append(rank)
            perm = rank[r_of_q]
            cols = bT[:, g * CHUNK : (g + 1) * CHUNK][:, perm]
            cpair.append(cch[rank.reshape(PAIR, 2)].mean(axis=1).astype(np.float32))
            for kp in range(kp_n):
                full = pair_pack(np.ascontiguousarray(cols), kp)  # [128, 2*CHUNK]
                if g == 0:
                    im[f"bt0p{kp}a"] = np.ascontiguousarray(
                        np.concatenate(
                            [full[:, 0:half], full[:, CHUNK : CHUNK + half]], axis=1
                        )
                    )
                    im[f"bt0p{kp}b"] = np.ascontiguousarray(
                        np.concatenate(
                            [full[:, half:CHUNK], full[:, CHUNK + half :]], axis=1
                        )
                    )
                else:
                    im[f"bt{g}p{kp}"] = full
        in_maps.append(im)
        all_ranks.append(ranks)
        all_cp.append(cpair)
    return in_maps, all_ranks, all_cp


def merge_results(a, b, n, b_batch_size, results, all_ranks, all_cp):
    """Subtract the fp32 pair bias from each chunk's 1024 pair-maxima, keep
    the top-NSEL pairs per (query, chunk), expand each to its 2 c-adjacent
    columns, refine with the exact fp32 reference distance, pick final top-n
    (ties -> lowest index), apply the reference's bookkeeping."""
    nseg = NB_SHARD // CHUNK
    na = a.shape[0]
    cand_parts = []
    for core in range(NCORES):
        oq = results[core]["out_q"].astype(np.float32)  # [NA, nseg*PAIR]
        for s in range(nseg):
            adj = oq[:, s * PAIR : (s + 1) * PAIR] - all_cp[core][s][None, :]
            top = np.argpartition(-adj, NSEL, axis=1)[:, :NSEL]
            rank = all_ranks[core][s]
            base = core * NB_SHARD + s * CHUNK
            for k in range(2):
                cand_parts.append(rank[2 * top + k] + base)
    cand = np.concatenate(cand_parts, axis=1)  # [NA, NCORES*nseg*NSEL*2]

    a2 = np.sum(a * a, axis=1)
    sa = np.sum(a, axis=1)
    b2 = np.sum(b * b, axis=1)
    sb = np.sum(b, axis=1)
    d = a.shape[1]
    out = np.empty((na, n), dtype=np.int64)
    CHQ = 128
    eps = np.float32(EPS)
    for q0 in range(0, na, CHQ):
        q1 = min(q0 + CHQ, na)
        Cc = cand[q0:q1]
        Bc = b[Cc]
        cross = np.matmul(Bc, a[q0:q1, :, None])[..., 0].astype(np.float32)
        sq = (
            a2[q0:q1, None]
            + b2[Cc]
            - np.float32(2.0) * cross
            + np.float32(2.0) * eps * (sa[q0:q1, None] - sb[Cc])
            + np.float32(d) * eps * eps
        )
        dist = np.sqrt(np.maximum(sq, np.float32(0.0)))
        ordr = np.lexsort((Cc, dist), axis=1)[:, :n]
        rows = np.arange(q1 - q0)[:, None]
        out[q0:q1] = Cc[rows, ordr]
    buggy = (out % b_batch_size) + (out // b_batch_size)
    return buggy.astype(np.int32)


def kernel(a, b, n, b_batch_size, trace=False):
    from concourse.bass_utils import run_bass_kernel_spmd

    a = np.ascontiguousarray(np.asarray(a, dtype=np.float32))
    b = np.ascontiguousarray(np.asarray(b, dtype=np.float32))
    n = int(n)
    b_batch_size = int(b_batch_size)

    nc = build_kernel()
    in_maps, all_ranks, all_cp = make_in_maps(a, b)
    res = run_bass_kernel_spmd(
        nc, in_maps, core_ids=list(range(NCORES)), trace=trace
    )
    out = merge_results(a, b, n, b_batch_size, res.results, all_ranks, all_cp)
    if trace:
        return out, res
    return out


# revision 24
# speedup vs baseline: 1.0110x; 1.0056x over previous
"""Sharded kNN (ArgDistanceMeasure) on 8 TRN2 NeuronCores.

~131.5us HW exec at full clock (vs 157us baseline; note the board runs some
executions at ~5/6 clock — warm N=512 matmul 454ns instead of 379ns — which
inflates any measurement by ~20%).

Strategy (FAISS-style sharded kNN):
  - b (the database, [65536, 512]) is sharded row-wise across 8 cores
    (8192 rows each); a (queries, [2048, 512]) is replicated.
  - Ranking identity: argmin_j ||a_i - b_j + eps||^2 over j only needs the
    column-dependent part  score[i,j] = 2*a_i.b_j - c_j  (maximized), where
    c_j = ||b_j||^2 - 2*eps*sum(b_j).  Row-constant terms don't affect
    per-row ranking.
  - Columns of each 2048-wide chunk are host-permuted so that device
    position q holds the column with c-sorted rank (q % 1024)*2 + (q // 1024):
    one pairwise-max over the chunk halves groups PAIRS of c-adjacent columns
    (spread <= 2 ranks), so the c_j bias can be applied on the host to the
    1024 pair-maxima — the PE runs a pure GEMM.
  - Per [128 queries x 2048 cols] chunk:
      PE:  fp8-e4m3 DoubleRow GEMM (K=256/matmul) accumulating 2*cross into
           PSUM; 8 matmuls of N=512 (~1.73us/chunk warm).
      ACT: two 512-col copies PSUM[0:1024] -> SBUF fp16 (s16a).
      DVE: two 512-col tensor_max ops, each reading one s16a half (SBUF
           fp16) and one PSUM fp32 quadrant directly (dual-PSUM-operand TT
           is rejected by walrus; one PSUM operand is fine) -> m2 fp16
           pair-maxima.  Each PSUM quadrant has exactly ONE consumer
           (ACT-a, ACT-b, L1a, L1b), so every next-next-chunk matmul
           carries a single semaphore wait, and the quadrants free
           in a staggered pipeline well before the PE needs them.
      DMA: m2 [128, 1024] fp16 (256KB/chunk) -> DRAM out_q.
  - Startup: the first m-slice of a (queries 0-511) and the first half of
    b-chunk 0 are small dedicated DMAs so the first matmul issues ~4us
    earlier than a monolithic preload; remaining loads are dependency-gated
    behind the critical wave.  Dummy matmuls on memset scratch warm the PE
    HAM clock (cold 1.2GHz -> warm 2.4GHz) while the first DMAs land.
  - Host: subtracts the fp32 per-pair bias from the 1024 pair-maxima per
    chunk, keeps the top-32 pairs per (query, chunk), expands each to its 2
    c-adjacent columns (2048 candidates/query), recomputes the exact fp32
    reference distance and picks the final top-n with the reference's
    tie-break and buggy index bookkeeping.  (Validated in simulation:
    0/32768 mismatches.)
"""
import numpy as np

NA, D, NB = 2048, 512, 65536
NCORES = 8
NB_SHARD = NB // NCORES  # 8192
CHUNK = 2048             # chunk width (4 PSUM banks)
PAIR = CHUNK // 2        # 1024 pair-maxima per chunk
NSEL = 32                # pairs kept per (query, chunk) on the host
NDUMMY = 7               # PE warmup matmuls (N=512) on scratch: span the
                         # first-DMA wait AND the ~3.4us HAM activity window,
                         # so real matmuls start warm (2.4GHz) as soon as
                         # their data lands
EPS = 1e-6
M0 = 4                   # m-tiles covered by the first a m-slice


def build_kernel(na=NA, nb_shard=NB_SHARD, chunk=CHUNK):
    import concourse.mybir as mybir
    from concourse import bacc
    from concourse.tile import TileContext, add_dep_helper

    FP8 = mybir.dt.float8e4
    F16 = mybir.dt.float16
    F32 = mybir.dt.float32
    DR = mybir.MatmulPerfMode.DoubleRow

    nseg = nb_shard // chunk
    kt = D // 128            # 4 K-tiles of 128
    kp_n = kt // 2           # 2 DoubleRow pairs (K=256 each)
    mt = na // 128

    nc = bacc.Bacc()

    # DoubleRow operands are [128, 2, cols] (two K-slices packed per
    # partition).  a is split into a first m-slice (queries 0..511) and the
    # rest; b chunk 0 is split into halves so the first matmul's operands are
    # small, early DMAs.
    atm0_p = [
        nc.declare_dram_parameter(f"atm0p{kp}", [128, 2 * 128 * M0], FP8, isOutput=False)
        for kp in range(kp_n)
    ]
    atr_p = [
        nc.declare_dram_parameter(
            f"atrp{kp}", [128, 2 * 128 * (mt - M0)], FP8, isOutput=False
        )
        for kp in range(kp_n)
    ]
    bt0h_p = [
        [
            nc.declare_dram_parameter(
                f"bt0p{kp}{h}", [128, 2 * (chunk // 2)], FP8, isOutput=False
            )
            for h in ("a", "b")
        ]
        for kp in range(kp_n)
    ]
    bt_p = {
        (g, kp): nc.declare_dram_parameter(
            f"bt{g}p{kp}", [128, 2 * chunk], FP8, isOutput=False
        )
        for g in range(1, nseg)
        for kp in range(kp_n)
    }
    out_q = nc.declare_dram_parameter("out_q", [na, nseg * PAIR], F16, isOutput=True)

    with TileContext(nc) as tc:
        with (
            tc.tile_pool(name="weights", bufs=1) as wpool,
            tc.tile_pool(name="psum", bufs=2, space="PSUM") as ppool,
            tc.tile_pool(name="scores", bufs=4) as spool,
            tc.tile_pool(name="pairs", bufs=8) as mpool,
        ):
            # --- PE warmup: dummy DoubleRow matmuls on memset scratch ---
            wscr = wpool.tile([128, 2 * 512], FP8, tag="wscr", name="wscr")
            nc.gpsimd.memset(wscr, 0)
            w3 = wscr.rearrange("p (i c) -> p i c", i=2)
            # Per-j PSUM tiles (4 tags x bufs=2 x 1 bank = all 8 banks): each
            # consumer (ACT-a<-j0, ACT-b<-j1, L1a<-j2, L1b<-j3) then depends
            # on just its own quadrant's two matmuls, so copies start ~1us
            # before the chunk's last matmul and the PE never waits on the
            # (tile-granular) PSUM WAR chain.
            ps_first = ppool.tile([128, 512], F32, tag="score0", name="ps00")
            for _ in range(NDUMMY):
                nc.tensor.matmul(
                    ps_first, w3[:, :, :128], w3,
                    start=True, stop=True, perf_mode=DR,
                )

            # --- critical first wave: a m-slice 0, b chunk 0 halves.
            # The chunk-0 halves are separate CONTIGUOUS tiles: a DMA into a
            # strided [128,2,1024] view of one big tile generates 1KB packets
            # and runs ~4x slower than these 2KB-per-partition lines.
            atm0 = []
            bt0h = [[None, None] for _ in range(kp_n)]
            half = chunk // 2
            for kp in range(kp_n):
                for hh in range(2):
                    bt0h[kp][hh] = wpool.tile(
                        [128, 2 * half], FP8, tag=f"bt0p{kp}h{hh}", name=f"bt0p{kp}h{hh}"
                    )
            crit0 = nc.sync.dma_start(out=bt0h[0][0], in_=bt0h_p[0][0][:, :])
            for kp in range(kp_n):
                t = wpool.tile(
                    [128, 2 * 128 * M0], FP8, tag=f"atm0p{kp}", name=f"atm0p{kp}"
                )
                nc.sync.dma_start(out=t, in_=atm0_p[kp][:, :])
                atm0.append(t)
            nc.sync.dma_start(out=bt0h[1][0], in_=bt0h_p[1][0][:, :])
            for kp in range(kp_n):
                crit_dma = nc.sync.dma_start(out=bt0h[kp][1], in_=bt0h_p[kp][1][:, :])
            bt0h3 = [
                [t.rearrange("p (i c) -> p i c", i=2) for t in row] for row in bt0h
            ]
            # --- gated preloads: rest of a, b chunks 1..3 ---
            atr = []
            for kp in range(kp_n):
                t = wpool.tile(
                    [128, 2 * 128 * (mt - M0)], FP8, tag=f"atrp{kp}", name=f"atrp{kp}"
                )
                d = nc.sync.dma_start(out=t, in_=atr_p[kp][:, :])
                add_dep_helper(d.ins, crit_dma.ins, True, "preload priority")
                atr.append(t)
            bt_t = {}
            for g in range(1, nseg):
                for kp in range(kp_n):
                    t = wpool.tile(
                        [128, 2 * chunk], FP8, tag=f"bt{g}p{kp}", name=f"bt{g}p{kp}"
                    )
                    d = nc.sync.dma_start(out=t, in_=bt_p[(g, kp)][:, :])
                    add_dep_helper(d.ins, crit_dma.ins, True, "preload priority")
                    bt_t[(g, kp)] = t

            atm0_3 = [t.rearrange("p (i c) -> p i c", i=2) for t in atm0]
            atr_3 = [t.rearrange("p (i c) -> p i c", i=2) for t in atr]
            bt3 = {
                (g, kp): bt_t[(g, kp)].rearrange("p (i c) -> p i c", i=2)
                for g in range(1, nseg)
                for kp in range(kp_n)
            }

            for s in range(nseg):
                for m in range(mt):
                    ps_j = []
                    for j in range(4):
                        if s == 0 and m == 0 and j == 0:
                            ps_j.append(ps_first)
                        else:
                            ps_j.append(
                                ppool.tile(
                                    [128, 512], F32, tag=f"score{j}", name=f"ps{j}"
                                )
                            )
                    # j grouped in halves so chunk 0's matmuls chase the
                    # half-DMAs; kp inner per half for PSUM accumulate.
                    for h in range(2):
                        for kp in range(kp_n):
                            if m < M0:
                                lhsT3 = atm0_3[kp][:, :, m * 128 : (m + 1) * 128]
                            else:
                                lhsT3 = atr_3[kp][:, :, (m - M0) * 128 : (m - M0 + 1) * 128]
                            for j in (2 * h, 2 * h + 1):
                                if s == 0:
                                    rhs3 = bt0h3[kp][h][:, :, (j % 2) * 512 : (j % 2 + 1) * 512]
                                else:
                                    rhs3 = bt3[(s, kp)][:, :, j * 512 : (j + 1) * 512]
                                nc.tensor.matmul(
                                    ps_j[j],
                                    lhsT3,
                                    rhs3,
                                    start=(kp == 0),
                                    stop=(kp == kp_n - 1),
                                    perf_mode=DR,
                                )
                    s16a = spool.tile([128, PAIR], F16, tag="s16a", name="s16a")
                    nc.scalar.copy(out=s16a[:, :512], in_=ps_j[0])
                    nc.scalar.copy(out=s16a[:, 512:], in_=ps_j[1])
                    m2 = mpool.tile([128, PAIR], F16, tag="m2", name="m2")
                    nc.vector.tensor_max(m2[:, :512], s16a[:, :512], ps_j[2])
                    nc.vector.tensor_max(m2[:, 512:], s16a[:, 512:], ps_j[3])
                    nc.sync.dma_start(
                        out=out_q[m * 128 : (m + 1) * 128, s * PAIR : (s + 1) * PAIR],
                        in_=m2,
                    )
    nc.compile()
    return nc


def make_in_maps(a, b):
    """Pack per-core inputs.  Columns of each 2048-wide chunk are permuted so
    that device position q holds the column with c-sorted rank
    (q % 1024)*2 + (q // 1024) — making the pairwise-max mates c-adjacent
    (spread <= 2 ranks), so the bias can be applied on the host to the 1024
    pair-maxima.  Returns (in_maps, ranks, cpair) where ranks[core][s][r] is
    the local column with the r-th smallest c and cpair[core][s][t] the mean
    c of pair t."""
    import ml_dtypes

    kt = D // 128
    kp_n = kt // 2
    aT8 = (2.0 * a).T.astype(ml_dtypes.float8_e4m3)   # [512, NA]
    bT8 = b.T.astype(ml_dtypes.float8_e4m3)           # [512, NB]
    b2 = np.sum(b * b, axis=1)
    sb = b.sum(axis=1)
    c = (b2 - np.float32(2.0 * EPS) * sb).astype(np.float32)
    nseg = NB_SHARD // CHUNK
    q = np.arange(CHUNK)
    r_of_q = (q % PAIR) * 2 + (q // PAIR)
    half = CHUNK // 2

    def pair_pack(mat, kp):
        # [128, 2*cols]: slot i holds K-tile (kp*2+i) rows of mat
        lo = mat[(kp * 2) * 128 : (kp * 2 + 1) * 128, :]
        hi = mat[(kp * 2 + 1) * 128 : (kp * 2 + 2) * 128, :]
        return np.ascontiguousarray(np.concatenate([lo, hi], axis=1))

    in_maps = []
    all_ranks = []
    all_cp = []
    for core in range(NCORES):
        sl = slice(core * NB_SHARD, (core + 1) * NB_SHARD)
        bT = bT8[:, sl]
        c_core = c[core * NB_SHARD : (core + 1) * NB_SHARD]
        ranks = []
        cpair = []
        im = {}
        for kp in range(kp_n):
            im[f"atm0p{kp}"] = pair_pack(aT8[:, : 128 * M0], kp)
            im[f"atrp{kp}"] = pair_pack(aT8[:, 128 * M0 :], kp)
        for g in range(nseg):
            cch = c_core[g * CHUNK : (g + 1) * CHUNK]
            rank = np.argsort(cch, kind="stable")
            ranks.append(rank)
            perm = rank[r_of_q]
            cols = bT[:, g * CHUNK : (g + 1) * CHUNK][:, perm]
            cpair.append(cch[rank.reshape(PAIR, 2)].mean(axis=1).astype(np.float32))
            for kp in range(kp_n):
                full = pair_pack(np.ascontiguousarray(cols), kp)  # [128, 2*CHUNK]
                if g == 0:
                    im[f"bt0p{kp}a"] = np.ascontiguousarray(
                        np.concatenate(
                            [full[:, 0:half], full[:, CHUNK : CHUNK + half]], axis=1
                        )
                    )
                    im[f"bt0p{kp}b"] = np.ascontiguousarray(
                        np.concatenate(
                            [full[:, half:CHUNK], full[:, CHUNK + half :]], axis=1
                        )
                    )
                else:
                    im[f"bt{g}p{kp}"] = full
        in_maps.append(im)
        all_ranks.append(ranks)
        all_cp.append(cpair)
    return in_maps, all_ranks, all_cp


def merge_results(a, b, n, b_batch_size, results, all_ranks, all_cp):
    """Subtract the fp32 pair bias from each chunk's 1024 pair-maxima, keep
    the top-NSEL pairs per (query, chunk), expand each to its 2 c-adjacent
    columns, refine with the exact fp32 reference distance, pick final top-n
    (ties -> lowest index), apply the reference's bookkeeping."""
    nseg = NB_SHARD // CHUNK
    na = a.shape[0]
    cand_parts = []
    for core in range(NCORES):
        oq = results[core]["out_q"].astype(np.float32)  # [NA, nseg*PAIR]
        for s in range(nseg):
            adj = oq[:, s * PAIR : (s + 1) * PAIR] - all_cp[core][s][None, :]
            top = np.argpartition(-adj, NSEL, axis=1)[:, :NSEL]
            rank = all_ranks[core][s]
            base = core * NB_SHARD + s * CHUNK
            for k in range(2):
                cand_parts.append(rank[2 * top + k] + base)
    cand = np.concatenate(cand_parts, axis=1)  # [NA, NCORES*nseg*NSEL*2]

    a2 = np.sum(a * a, axis=1)
    sa = np.sum(a, axis=1)
    b2 = np.sum(b * b, axis=1)
    sb = np.sum(b, axis=1)
    d = a.shape[1]
    out = np.empty((na, n), dtype=np.int64)
    CHQ = 128
    eps = np.float32(EPS)
    for q0 in range(0, na, CHQ):
        q1 = min(q0 + CHQ, na)
        Cc = cand[q0:q1]
        Bc = b[Cc]
        cross = np.matmul(Bc, a[q0:q1, :, None])[..., 0].astype(np.float32)
        sq = (
            a2[q0:q1, None]
            + b2[Cc]
            - np.float32(2.0) * cross
            + np.float32(2.0) * eps * (sa[q0:q1, None] - sb[Cc])
            + np.float32(d) * eps * eps
        )
        dist = np.sqrt(np.maximum(sq, np.float32(0.0)))
        ordr = np.lexsort((Cc, dist), axis=1)[:, :n]
        rows = np.arange(q1 - q0)[:, None]
        out[q0:q1] = Cc[rows, ordr]
    buggy = (out % b_batch_size) + (out // b_batch_size)
    return buggy.astype(np.int32)


def kernel(a, b, n, b_batch_size, trace=False):
    from concourse.bass_utils import run_bass_kernel_spmd

    a = np.ascontiguousarray(np.asarray(a, dtype=np.float32))
    b = np.ascontiguousarray(np.asarray(b, dtype=np.float32))
    n = int(n)
    b_batch_size = int(b_batch_size)

    nc = build_kernel()
    in_maps, all_ranks, all_cp = make_in_maps(a, b)
    res = run_bass_kernel_spmd(
        nc, in_maps, core_ids=list(range(NCORES)), trace=trace
    )
    out = merge_results(a, b, n, b_batch_size, res.results, all_ranks, all_cp)
    if trace:
        return out, res
    return out


# revision 27
# speedup vs baseline: 1.0205x; 1.0094x over previous
"""Sharded kNN (ArgDistanceMeasure) on 8 TRN2 NeuronCores.

~131.5us HW exec at full clock (vs 157us baseline; note the board runs some
executions at ~5/6 clock — warm N=512 matmul 454ns instead of 379ns — which
inflates any measurement by ~20%).

Strategy (FAISS-style sharded kNN):
  - b (the database, [65536, 512]) is sharded row-wise across 8 cores
    (8192 rows each); a (queries, [2048, 512]) is replicated.
  - Ranking identity: argmin_j ||a_i - b_j + eps||^2 over j only needs the
    column-dependent part  score[i,j] = 2*a_i.b_j - c_j  (maximized), where
    c_j = ||b_j||^2 - 2*eps*sum(b_j).  Row-constant terms don't affect
    per-row ranking.
  - Columns of each 2048-wide chunk are host-permuted so that device
    position q holds the column with c-sorted rank (q % 1024)*2 + (q // 1024):
    one pairwise-max over the chunk halves groups PAIRS of c-adjacent columns
    (spread <= 2 ranks), so the c_j bias can be applied on the host to the
    1024 pair-maxima — the PE runs a pure GEMM.
  - Per [128 queries x 2048 cols] chunk:
      PE:  fp8-e4m3 DoubleRow GEMM (K=256/matmul) accumulating 2*cross into
           PSUM; 8 matmuls of N=512 (~1.73us/chunk warm).
      ACT: two 512-col copies PSUM[0:1024] -> SBUF fp16 (s16a).
      DVE: two 512-col tensor_max ops, each reading one s16a half (SBUF
           fp16) and one PSUM fp32 quadrant directly (dual-PSUM-operand TT
           is rejected by walrus; one PSUM operand is fine) -> m2 fp16
           pair-maxima.  Each PSUM quadrant has exactly ONE consumer
           (ACT-a, ACT-b, L1a, L1b), so every next-next-chunk matmul
           carries a single semaphore wait, and the quadrants free
           in a staggered pipeline well before the PE needs them.
      DMA: m2 [128, 1024] fp16 (256KB/chunk) -> DRAM out_q.
  - Startup: the first m-slice of a (queries 0-511) and the first half of
    b-chunk 0 are small dedicated DMAs so the first matmul issues ~4us
    earlier than a monolithic preload; remaining loads are dependency-gated
    behind the critical wave.  Dummy matmuls on memset scratch warm the PE
    HAM clock (cold 1.2GHz -> warm 2.4GHz) while the first DMAs land.
  - Host: subtracts the fp32 per-pair bias from the 1024 pair-maxima per
    chunk, keeps the top-32 pairs per (query, chunk), expands each to its 2
    c-adjacent columns (2048 candidates/query), recomputes the exact fp32
    reference distance and picks the final top-n with the reference's
    tie-break and buggy index bookkeeping.  (Validated in simulation:
    0/32768 mismatches.)
"""
import numpy as np

NA, D, NB = 2048, 512, 65536
NCORES = 8
NB_SHARD = NB // NCORES  # 8192
CHUNK = 2048             # chunk width (4 PSUM banks)
PAIR = CHUNK // 2        # 1024 pair-maxima per chunk
NSEL = 32                # pairs kept per (query, chunk) on the host
NDUMMY = 7               # PE warmup matmuls (N=512) on scratch: span the
                         # first-DMA wait AND the ~3.4us HAM activity window,
                         # so real matmuls start warm (2.4GHz) as soon as
                         # their data lands
EPS = 1e-6
M0 = 4                   # m-tiles covered by the first a m-slice


def build_kernel(na=NA, nb_shard=NB_SHARD, chunk=CHUNK):
    import concourse.mybir as mybir
    from concourse import bacc
    from concourse.tile import TileContext, add_dep_helper

    FP8 = mybir.dt.float8e4
    F16 = mybir.dt.float16
    F32 = mybir.dt.float32
    DR = mybir.MatmulPerfMode.DoubleRow

    nseg = nb_shard // chunk
    kt = D // 128            # 4 K-tiles of 128
    kp_n = kt // 2           # 2 DoubleRow pairs (K=256 each)
    mt = na // 128

    nc = bacc.Bacc()

    # DoubleRow operands are [128, 2, cols] (two K-slices packed per
    # partition).  a is split into a first m-slice (queries 0..511) and the
    # rest; b chunk 0 is split into halves so the first matmul's operands are
    # small, early DMAs.
    atm0_p = [
        nc.declare_dram_parameter(f"atm0p{kp}", [128, 2 * 128 * M0], FP8, isOutput=False)
        for kp in range(kp_n)
    ]
    atr_p = [
        nc.declare_dram_parameter(
            f"atrp{kp}", [128, 2 * 128 * (mt - M0)], FP8, isOutput=False
        )
        for kp in range(kp_n)
    ]
    bt0h_p = [
        [
            nc.declare_dram_parameter(
                f"bt0p{kp}{h}", [128, 2 * (chunk // 2)], FP8, isOutput=False
            )
            for h in ("a", "b")
        ]
        for kp in range(kp_n)
    ]
    bt_p = {
        (g, kp): nc.declare_dram_parameter(
            f"bt{g}p{kp}", [128, 2 * chunk], FP8, isOutput=False
        )
        for g in range(1, nseg)
        for kp in range(kp_n)
    }
    # Pair-maxima output, fp8 (selection margin validated: worst true-member
    # rank 5 of 32 kept), packed TWO m-chunks per row-block so each out-DMA
    # moves a [128, 2048] tile with contiguous 2KB per-partition lines:
    # row (m//2)*128 + r, col s*2048 + (m%2)*1024 + p  holds the value for
    # (query m*128+r, chunk s, pair p).  Halves output bytes and trigger
    # count vs per-chunk fp16 — the final out-DMA's completion gates the
    # kernel-end barrier, so queue backlog here is exec time.
    out_qp = nc.declare_dram_parameter(
        "out_qp", [(na // 256) * 128, nseg * 2 * PAIR], FP8, isOutput=True
    )

    with TileContext(nc) as tc:
        with (
            tc.tile_pool(name="weights", bufs=1) as wpool,
            tc.tile_pool(name="psum", bufs=2, space="PSUM") as ppool,
            tc.tile_pool(name="scores", bufs=4) as spool,
            tc.tile_pool(name="pairs", bufs=8) as mpool,
        ):
            # --- PE warmup: dummy DoubleRow matmuls on memset scratch ---
            wscr = wpool.tile([128, 2 * 512], FP8, tag="wscr", name="wscr")
            nc.gpsimd.memset(wscr, 0)
            w3 = wscr.rearrange("p (i c) -> p i c", i=2)
            # Per-j PSUM tiles (4 tags x bufs=2 x 1 bank = all 8 banks): each
            # consumer (ACT-a<-j0, ACT-b<-j1, L1a<-j2, L1b<-j3) then depends
            # on just its own quadrant's two matmuls, so copies start ~1us
            # before the chunk's last matmul and the PE never waits on the
            # (tile-granular) PSUM WAR chain.
            ps_first = ppool.tile([128, 512], F32, tag="score0", name="ps00")
            for _ in range(NDUMMY):
                nc.tensor.matmul(
                    ps_first, w3[:, :, :128], w3,
                    start=True, stop=True, perf_mode=DR,
                )

            # --- critical first wave: a m-slice 0, b chunk 0 halves.
            # The chunk-0 halves are separate CONTIGUOUS tiles: a DMA into a
            # strided [128,2,1024] view of one big tile generates 1KB packets
            # and runs ~4x slower than these 2KB-per-partition lines.
            atm0 = []
            bt0h = [[None, None] for _ in range(kp_n)]
            half = chunk // 2
            for kp in range(kp_n):
                for hh in range(2):
                    bt0h[kp][hh] = wpool.tile(
                        [128, 2 * half], FP8, tag=f"bt0p{kp}h{hh}", name=f"bt0p{kp}h{hh}"
                    )
            crit0 = nc.sync.dma_start(out=bt0h[0][0], in_=bt0h_p[0][0][:, :])
            for kp in range(kp_n):
                t = wpool.tile(
                    [128, 2 * 128 * M0], FP8, tag=f"atm0p{kp}", name=f"atm0p{kp}"
                )
                nc.sync.dma_start(out=t, in_=atm0_p[kp][:, :])
                atm0.append(t)
            nc.sync.dma_start(out=bt0h[1][0], in_=bt0h_p[1][0][:, :])
            for kp in range(kp_n):
                crit_dma = nc.sync.dma_start(out=bt0h[kp][1], in_=bt0h_p[kp][1][:, :])
            bt0h3 = [
                [t.rearrange("p (i c) -> p i c", i=2) for t in row] for row in bt0h
            ]
            # --- gated preloads: rest of a, b chunks 1..3 ---
            atr = []
            for kp in range(kp_n):
                t = wpool.tile(
                    [128, 2 * 128 * (mt - M0)], FP8, tag=f"atrp{kp}", name=f"atrp{kp}"
                )
                d = nc.sync.dma_start(out=t, in_=atr_p[kp][:, :])
                add_dep_helper(d.ins, crit_dma.ins, True, "preload priority")
                atr.append(t)
            bt_t = {}
            for g in range(1, nseg):
                for kp in range(kp_n):
                    t = wpool.tile(
                        [128, 2 * chunk], FP8, tag=f"bt{g}p{kp}", name=f"bt{g}p{kp}"
                    )
                    d = nc.sync.dma_start(out=t, in_=bt_p[(g, kp)][:, :])
                    add_dep_helper(d.ins, crit_dma.ins, True, "preload priority")
                    bt_t[(g, kp)] = t

            atm0_3 = [t.rearrange("p (i c) -> p i c", i=2) for t in atm0]
            atr_3 = [t.rearrange("p (i c) -> p i c", i=2) for t in atr]
            bt3 = {
                (g, kp): bt_t[(g, kp)].rearrange("p (i c) -> p i c", i=2)
                for g in range(1, nseg)
                for kp in range(kp_n)
            }

            for s in range(nseg):
                for m in range(mt):
                    ps_j = []
                    for j in range(4):
                        if s == 0 and m == 0 and j == 0:
                            ps_j.append(ps_first)
                        else:
                            ps_j.append(
                                ppool.tile(
                                    [128, 512], F32, tag=f"score{j}", name=f"ps{j}"
                                )
                            )
                    # j grouped in halves so chunk 0's matmuls chase the
                    # half-DMAs; kp inner per half for PSUM accumulate.
                    for h in range(2):
                        for kp in range(kp_n):
                            if m < M0:
                                lhsT3 = atm0_3[kp][:, :, m * 128 : (m + 1) * 128]
                            else:
                                lhsT3 = atr_3[kp][:, :, (m - M0) * 128 : (m - M0 + 1) * 128]
                            for j in (2 * h, 2 * h + 1):
                                if s == 0:
                                    rhs3 = bt0h3[kp][h][:, :, (j % 2) * 512 : (j % 2 + 1) * 512]
                                else:
                                    rhs3 = bt3[(s, kp)][:, :, j * 512 : (j + 1) * 512]
                                nc.tensor.matmul(
                                    ps_j[j],
                                    lhsT3,
                                    rhs3,
                                    start=(kp == 0),
                                    stop=(kp == kp_n - 1),
                                    perf_mode=DR,
                                )
                    s16a = spool.tile([128, PAIR], F16, tag="s16a", name="s16a")
                    nc.scalar.copy(out=s16a[:, :512], in_=ps_j[0])
                    nc.scalar.copy(out=s16a[:, 512:], in_=ps_j[1])
                    if m % 2 == 0:
                        m2 = mpool.tile([128, 2 * PAIR], FP8, tag="m2", name="m2")
                    off = (m % 2) * PAIR
                    nc.vector.tensor_max(
                        m2[:, off : off + 512], s16a[:, :512], ps_j[2]
                    )
                    nc.vector.tensor_max(
                        m2[:, off + 512 : off + PAIR], s16a[:, 512:], ps_j[3]
                    )
                    if m % 2 == 1:
                        mp_ = m // 2
                        nc.sync.dma_start(
                            out=out_qp[
                                mp_ * 128 : (mp_ + 1) * 128,
                                s * 2 * PAIR : (s + 1) * 2 * PAIR,
                            ],
                            in_=m2,
                        )
    nc.compile()
    return nc


def make_in_maps(a, b):
    """Pack per-core inputs.  Columns of each 2048-wide chunk are permuted so
    that device position q holds the column with c-sorted rank
    (q % 1024)*2 + (q // 1024) — making the pairwise-max mates c-adjacent
    (spread <= 2 ranks), so the bias can be applied on the host to the 1024
    pair-maxima.  Returns (in_maps, ranks, cpair) where ranks[core][s][r] is
    the local column with the r-th smallest c and cpair[core][s][t] the mean
    c of pair t."""
    import ml_dtypes

    kt = D // 128
    kp_n = kt // 2
    aT8 = (2.0 * a).T.astype(ml_dtypes.float8_e4m3)   # [512, NA]
    bT8 = b.T.astype(ml_dtypes.float8_e4m3)           # [512, NB]
    b2 = np.sum(b * b, axis=1)
    sb = b.sum(axis=1)
    c = (b2 - np.float32(2.0 * EPS) * sb).astype(np.float32)
    nseg = NB_SHARD // CHUNK
    q = np.arange(CHUNK)
    r_of_q = (q % PAIR) * 2 + (q // PAIR)
    half = CHUNK // 2

    def pair_pack(mat, kp):
        # [128, 2*cols]: slot i holds K-tile (kp*2+i) rows of mat
        lo = mat[(kp * 2) * 128 : (kp * 2 + 1) * 128, :]
        hi = mat[(kp * 2 + 1) * 128 : (kp * 2 + 2) * 128, :]
        return np.ascontiguousarray(np.concatenate([lo, hi], axis=1))

    in_maps = []
    all_ranks = []
    all_cp = []
    for core in range(NCORES):
        sl = slice(core * NB_SHARD, (core + 1) * NB_SHARD)
        bT = bT8[:, sl]
        c_core = c[core * NB_SHARD : (core + 1) * NB_SHARD]
        ranks = []
        cpair = []
        im = {}
        for kp in range(kp_n):
            im[f"atm0p{kp}"] = pair_pack(aT8[:, : 128 * M0], kp)
            im[f"atrp{kp}"] = pair_pack(aT8[:, 128 * M0 :], kp)
        for g in range(nseg):
            cch = c_core[g * CHUNK : (g + 1) * CHUNK]
            rank = np.argsort(cch, kind="stable")
            ranks.append(rank)
            perm = rank[r_of_q]
            cols = bT[:, g * CHUNK : (g + 1) * CHUNK][:, perm]
            cpair.append(cch[rank.reshape(PAIR, 2)].mean(axis=1).astype(np.float32))
            for kp in range(kp_n):
                full = pair_pack(np.ascontiguousarray(cols), kp)  # [128, 2*CHUNK]
                if g == 0:
                    im[f"bt0p{kp}a"] = np.ascontiguousarray(
                        np.concatenate(
                            [full[:, 0:half], full[:, CHUNK : CHUNK + half]], axis=1
                        )
                    )
                    im[f"bt0p{kp}b"] = np.ascontiguousarray(
                        np.concatenate(
                            [full[:, half:CHUNK], full[:, CHUNK + half :]], axis=1
                        )
                    )
                else:
                    im[f"bt{g}p{kp}"] = full
        in_maps.append(im)
        all_ranks.append(ranks)
        all_cp.append(cpair)
    return in_maps, all_ranks, all_cp


def merge_results(a, b, n, b_batch_size, results, all_ranks, all_cp):
    """Subtract the fp32 pair bias from each chunk's 1024 pair-maxima, keep
    the top-NSEL pairs per (query, chunk), expand each to its 2 c-adjacent
    columns, refine with the exact fp32 reference distance, pick final top-n
    (ties -> lowest index), apply the reference's bookkeeping."""
    nseg = NB_SHARD // CHUNK
    na = a.shape[0]
    cand_parts = []
    for core in range(NCORES):
        # Decode the 2-chunk-packed layout: [(na//256)*128, nseg*2*PAIR] ->
        # [na, nseg, PAIR] with query index q = (mp*2 + off)*128 + r.
        oq = (
            results[core]["out_qp"]
            .astype(np.float32)
            .reshape(na // 256, 128, nseg, 2, PAIR)
            .transpose(0, 3, 1, 2, 4)
            .reshape(na, nseg, PAIR)
        )
        for s in range(nseg):
            adj = oq[:, s, :] - all_cp[core][s][None, :]
            top = np.argpartition(-adj, NSEL, axis=1)[:, :NSEL]
            rank = all_ranks[core][s]
            base = core * NB_SHARD + s * CHUNK
            for k in range(2):
                cand_parts.append(rank[2 * top + k] + base)
    cand = np.concatenate(cand_parts, axis=1)  # [NA, NCORES*nseg*NSEL*2]

    a2 = np.sum(a * a, axis=1)
    sa = np.sum(a, axis=1)
    b2 = np.sum(b * b, axis=1)
    sb = np.sum(b, axis=1)
    d = a.shape[1]
    out = np.empty((na, n), dtype=np.int64)
    CHQ = 128
    eps = np.float32(EPS)
    for q0 in range(0, na, CHQ):
        q1 = min(q0 + CHQ, na)
        Cc = cand[q0:q1]
        Bc = b[Cc]
        cross = np.matmul(Bc, a[q0:q1, :, None])[..., 0].astype(np.float32)
        sq = (
            a2[q0:q1, None]
            + b2[Cc]
            - np.float32(2.0) * cross
            + np.float32(2.0) * eps * (sa[q0:q1, None] - sb[Cc])
            + np.float32(d) * eps * eps
        )
        dist = np.sqrt(np.maximum(sq, np.float32(0.0)))
        ordr = np.lexsort((Cc, dist), axis=1)[:, :n]
        rows = np.arange(q1 - q0)[:, None]
        out[q0:q1] = Cc[rows, ordr]
    buggy = (out % b_batch_size) + (out // b_batch_size)
    return buggy.astype(np.int32)


def kernel(a, b, n, b_batch_size, trace=False):
    from concourse.bass_utils import run_bass_kernel_spmd

    a = np.ascontiguousarray(np.asarray(a, dtype=np.float32))
    b = np.ascontiguousarray(np.asarray(b, dtype=np.float32))
    n = int(n)
    b_batch_size = int(b_batch_size)

    nc = build_kernel()
    in_maps, all_ranks, all_cp = make_in_maps(a, b)
    res = run_bass_kernel_spmd(
        nc, in_maps, core_ids=list(range(NCORES)), trace=trace
    )
    out = merge_results(a, b, n, b_batch_size, res.results, all_ranks, all_cp)
    if trace:
        return out, res
    return out


# revision 29
# speedup vs baseline: 1.0248x; 1.0041x over previous
"""Sharded kNN (ArgDistanceMeasure) on 8 TRN2 NeuronCores.

~131.5us HW exec at full clock (vs 157us baseline; note the board runs some
executions at ~5/6 clock — warm N=512 matmul 454ns instead of 379ns — which
inflates any measurement by ~20%).

Strategy (FAISS-style sharded kNN):
  - b (the database, [65536, 512]) is sharded row-wise across 8 cores
    (8192 rows each); a (queries, [2048, 512]) is replicated.
  - Ranking identity: argmin_j ||a_i - b_j + eps||^2 over j only needs the
    column-dependent part  score[i,j] = 2*a_i.b_j - c_j  (maximized), where
    c_j = ||b_j||^2 - 2*eps*sum(b_j).  Row-constant terms don't affect
    per-row ranking.
  - Columns of each 2048-wide chunk are host-permuted so that device
    position q holds the column with c-sorted rank (q % 1024)*2 + (q // 1024):
    one pairwise-max over the chunk halves groups PAIRS of c-adjacent columns
    (spread <= 2 ranks), so the c_j bias can be applied on the host to the
    1024 pair-maxima — the PE runs a pure GEMM.
  - Per [128 queries x 2048 cols] chunk:
      PE:  fp8-e4m3 DoubleRow GEMM (K=256/matmul) accumulating 2*cross into
           PSUM; 8 matmuls of N=512 (~1.73us/chunk warm).
      ACT: two 512-col copies PSUM[0:1024] -> SBUF fp16 (s16a).
      DVE: two 512-col tensor_max ops, each reading one s16a half (SBUF
           fp16) and one PSUM fp32 quadrant directly (dual-PSUM-operand TT
           is rejected by walrus; one PSUM operand is fine) -> m2 fp16
           pair-maxima.  Each PSUM quadrant has exactly ONE consumer
           (ACT-a, ACT-b, L1a, L1b), so every next-next-chunk matmul
           carries a single semaphore wait, and the quadrants free
           in a staggered pipeline well before the PE needs them.
      DMA: m2 [128, 1024] fp16 (256KB/chunk) -> DRAM out_q.
  - Startup: the first m-slice of a (queries 0-511) and the first half of
    b-chunk 0 are small dedicated DMAs so the first matmul issues ~4us
    earlier than a monolithic preload; remaining loads are dependency-gated
    behind the critical wave.  Dummy matmuls on memset scratch warm the PE
    HAM clock (cold 1.2GHz -> warm 2.4GHz) while the first DMAs land.
  - Host: subtracts the fp32 per-pair bias from the 1024 pair-maxima per
    chunk, keeps the top-32 pairs per (query, chunk), expands each to its 2
    c-adjacent columns (2048 candidates/query), recomputes the exact fp32
    reference distance and picks the final top-n with the reference's
    tie-break and buggy index bookkeeping.  (Validated in simulation:
    0/32768 mismatches.)
"""
import numpy as np

NA, D, NB = 2048, 512, 65536
NCORES = 8
NB_SHARD = NB // NCORES  # 8192
CHUNK = 2048             # chunk width (4 PSUM banks)
PAIR = CHUNK // 2        # 1024 pair-maxima per chunk
NSEL = 32                # pairs kept per (query, chunk) on the host
NDUMMY = 7               # PE warmup matmuls (N=512) on scratch: span the
                         # first-DMA wait AND the ~3.4us HAM activity window,
                         # so real matmuls start warm (2.4GHz) as soon as
                         # their data lands
EPS = 1e-6
M0 = 4                   # m-tiles covered by the first a m-slice


def build_kernel(na=NA, nb_shard=NB_SHARD, chunk=CHUNK):
    import concourse.mybir as mybir
    from concourse import bacc
    from concourse.tile import TileContext, add_dep_helper

    FP8 = mybir.dt.float8e4
    F16 = mybir.dt.float16
    F32 = mybir.dt.float32
    DR = mybir.MatmulPerfMode.DoubleRow

    nseg = nb_shard // chunk
    kt = D // 128            # 4 K-tiles of 128
    kp_n = kt // 2           # 2 DoubleRow pairs (K=256 each)
    mt = na // 128

    nc = bacc.Bacc()

    # DoubleRow operands are [128, 2, cols] (two K-slices packed per
    # partition).  a is split into a first m-slice (queries 0..511) and the
    # rest; b chunk 0 is split into halves so the first matmul's operands are
    # small, early DMAs.
    atm0_p = [
        nc.declare_dram_parameter(f"atm0p{kp}", [128, 2 * 128 * M0], FP8, isOutput=False)
        for kp in range(kp_n)
    ]
    atr_p = [
        nc.declare_dram_parameter(
            f"atrp{kp}", [128, 2 * 128 * (mt - M0)], FP8, isOutput=False
        )
        for kp in range(kp_n)
    ]
    bt0h_p = [
        [
            nc.declare_dram_parameter(
                f"bt0p{kp}{h}", [128, 2 * (chunk // 2)], FP8, isOutput=False
            )
            for h in ("a", "b")
        ]
        for kp in range(kp_n)
    ]
    bt_p = {
        (g, kp): nc.declare_dram_parameter(
            f"bt{g}p{kp}", [128, 2 * chunk], FP8, isOutput=False
        )
        for g in range(1, nseg)
        for kp in range(kp_n)
    }
    # Pair-maxima output, fp8 (selection margin validated: worst true-member
    # rank 5 of 32 kept), packed TWO m-chunks per row-block so each out-DMA
    # moves a [128, 2048] tile with contiguous 2KB per-partition lines:
    # row (m//2)*128 + r, col s*2048 + (m%2)*1024 + p  holds the value for
    # (query m*128+r, chunk s, pair p).  Halves output bytes and trigger
    # count vs per-chunk fp16 — the final out-DMA's completion gates the
    # kernel-end barrier, so queue backlog here is exec time.
    out_qp = nc.declare_dram_parameter(
        "out_qp", [(na // 256) * 128, nseg * 2 * PAIR], FP8, isOutput=True
    )

    with TileContext(nc) as tc:
        with (
            tc.tile_pool(name="weights", bufs=1) as wpool,
            tc.tile_pool(name="psum", bufs=2, space="PSUM") as ppool,
            tc.tile_pool(name="scores", bufs=4) as spool,
            tc.tile_pool(name="pairs", bufs=8) as mpool,
        ):
            # --- PE warmup: dummy DoubleRow matmuls on memset scratch ---
            wscr = wpool.tile([128, 2 * 512], FP8, tag="wscr", name="wscr")
            nc.gpsimd.memset(wscr, 0)
            w3 = wscr.rearrange("p (i c) -> p i c", i=2)
            # Per-j PSUM tiles (4 tags x bufs=2 x 1 bank = all 8 banks): each
            # consumer (ACT-a<-j0, ACT-b<-j1, L1a<-j2, L1b<-j3) then depends
            # on just its own quadrant's two matmuls, so copies start ~1us
            # before the chunk's last matmul and the PE never waits on the
            # (tile-granular) PSUM WAR chain.
            ps_first = ppool.tile([128, 512], F32, tag="score0", name="ps00")
            for _ in range(NDUMMY):
                nc.tensor.matmul(
                    ps_first, w3[:, :, :128], w3,
                    start=True, stop=True, perf_mode=DR,
                )

            # --- critical first wave: a m-slice 0, b chunk 0 halves.
            # The chunk-0 halves are separate CONTIGUOUS tiles: a DMA into a
            # strided [128,2,1024] view of one big tile generates 1KB packets
            # and runs ~4x slower than these 2KB-per-partition lines.
            atm0 = []
            bt0h = [[None, None] for _ in range(kp_n)]
            half = chunk // 2
            for kp in range(kp_n):
                for hh in range(2):
                    bt0h[kp][hh] = wpool.tile(
                        [128, 2 * half], FP8, tag=f"bt0p{kp}h{hh}", name=f"bt0p{kp}h{hh}"
                    )
            crit0 = nc.sync.dma_start(out=bt0h[0][0], in_=bt0h_p[0][0][:, :])
            for kp in range(kp_n):
                t = wpool.tile(
                    [128, 2 * 128 * M0], FP8, tag=f"atm0p{kp}", name=f"atm0p{kp}"
                )
                nc.sync.dma_start(out=t, in_=atm0_p[kp][:, :])
                atm0.append(t)
            nc.sync.dma_start(out=bt0h[1][0], in_=bt0h_p[1][0][:, :])
            for kp in range(kp_n):
                crit_dma = nc.sync.dma_start(out=bt0h[kp][1], in_=bt0h_p[kp][1][:, :])
            bt0h3 = [
                [t.rearrange("p (i c) -> p i c", i=2) for t in row] for row in bt0h
            ]
            # --- gated preloads: rest of a, b chunks 1..3 ---
            atr = []
            for kp in range(kp_n):
                t = wpool.tile(
                    [128, 2 * 128 * (mt - M0)], FP8, tag=f"atrp{kp}", name=f"atrp{kp}"
                )
                d = nc.sync.dma_start(out=t, in_=atr_p[kp][:, :])
                add_dep_helper(d.ins, crit_dma.ins, True, "preload priority")
                atr.append(t)
            # bt1 rides with atr behind the critical wave; bt2/bt3 are a
            # SECOND gated wave behind bt1 so the out-DMA packets (whose
            # final completion gates the kernel-end barrier) start draining
            # mid-stream instead of queueing behind 4MB of preloads.
            bt_t = {}
            wave1 = None
            for g in range(1, nseg):
                for kp in range(kp_n):
                    t = wpool.tile(
                        [128, 2 * chunk], FP8, tag=f"bt{g}p{kp}", name=f"bt{g}p{kp}"
                    )
                    gate = crit_dma if g == 1 else wave1
                    d = nc.sync.dma_start(out=t, in_=bt_p[(g, kp)][:, :])
                    add_dep_helper(d.ins, gate.ins, True, "preload priority")
                    if g == 1:
                        wave1 = d
                    bt_t[(g, kp)] = t

            atm0_3 = [t.rearrange("p (i c) -> p i c", i=2) for t in atm0]
            atr_3 = [t.rearrange("p (i c) -> p i c", i=2) for t in atr]
            bt3 = {
                (g, kp): bt_t[(g, kp)].rearrange("p (i c) -> p i c", i=2)
                for g in range(1, nseg)
                for kp in range(kp_n)
            }

            for s in range(nseg):
                for m in range(mt):
                    ps_j = []
                    for j in range(4):
                        if s == 0 and m == 0 and j == 0:
                            ps_j.append(ps_first)
                        else:
                            ps_j.append(
                                ppool.tile(
                                    [128, 512], F32, tag=f"score{j}", name=f"ps{j}"
                                )
                            )
                    # j grouped in halves so chunk 0's matmuls chase the
                    # half-DMAs; kp inner per half for PSUM accumulate.
                    for h in range(2):
                        for kp in range(kp_n):
                            if m < M0:
                                lhsT3 = atm0_3[kp][:, :, m * 128 : (m + 1) * 128]
                            else:
                                lhsT3 = atr_3[kp][:, :, (m - M0) * 128 : (m - M0 + 1) * 128]
                            for j in (2 * h, 2 * h + 1):
                                if s == 0:
                                    rhs3 = bt0h3[kp][h][:, :, (j % 2) * 512 : (j % 2 + 1) * 512]
                                else:
                                    rhs3 = bt3[(s, kp)][:, :, j * 512 : (j + 1) * 512]
                                nc.tensor.matmul(
                                    ps_j[j],
                                    lhsT3,
                                    rhs3,
                                    start=(kp == 0),
                                    stop=(kp == kp_n - 1),
                                    perf_mode=DR,
                                )
                    s16a = spool.tile([128, PAIR], F16, tag="s16a", name="s16a")
                    nc.scalar.copy(out=s16a[:, :512], in_=ps_j[0])
                    nc.scalar.copy(out=s16a[:, 512:], in_=ps_j[1])
                    if m % 2 == 0:
                        m2 = mpool.tile([128, 2 * PAIR], FP8, tag="m2", name="m2")
                    off = (m % 2) * PAIR
                    nc.vector.tensor_max(
                        m2[:, off : off + 512], s16a[:, :512], ps_j[2]
                    )
                    nc.vector.tensor_max(
                        m2[:, off + 512 : off + PAIR], s16a[:, 512:], ps_j[3]
                    )
                    last_pair = s == nseg - 1 and m >= mt - 2
                    if last_pair:
                        # Final pair: one half-DMA per chunk so the kernel-end
                        # barrier only waits on a 128KB transfer.
                        mp_ = m // 2
                        nc.sync.dma_start(
                            out=out_qp[
                                mp_ * 128 : (mp_ + 1) * 128,
                                s * 2 * PAIR + off : s * 2 * PAIR + off + PAIR,
                            ],
                            in_=m2[:, off : off + PAIR],
                        )
                    elif m % 2 == 1:
                        mp_ = m // 2
                        nc.sync.dma_start(
                            out=out_qp[
                                mp_ * 128 : (mp_ + 1) * 128,
                                s * 2 * PAIR : (s + 1) * 2 * PAIR,
                            ],
                            in_=m2,
                        )
    nc.compile()
    return nc


def make_in_maps(a, b):
    """Pack per-core inputs.  Columns of each 2048-wide chunk are permuted so
    that device position q holds the column with c-sorted rank
    (q % 1024)*2 + (q // 1024) — making the pairwise-max mates c-adjacent
    (spread <= 2 ranks), so the bias can be applied on the host to the 1024
    pair-maxima.  Returns (in_maps, ranks, cpair) where ranks[core][s][r] is
    the local column with the r-th smallest c and cpair[core][s][t] the mean
    c of pair t."""
    import ml_dtypes

    kt = D // 128
    kp_n = kt // 2
    aT8 = (2.0 * a).T.astype(ml_dtypes.float8_e4m3)   # [512, NA]
    bT8 = b.T.astype(ml_dtypes.float8_e4m3)           # [512, NB]
    b2 = np.sum(b * b, axis=1)
    sb = b.sum(axis=1)
    c = (b2 - np.float32(2.0 * EPS) * sb).astype(np.float32)
    nseg = NB_SHARD // CHUNK
    q = np.arange(CHUNK)
    r_of_q = (q % PAIR) * 2 + (q // PAIR)
    half = CHUNK // 2

    def pair_pack(mat, kp):
        # [128, 2*cols]: slot i holds K-tile (kp*2+i) rows of mat
        lo = mat[(kp * 2) * 128 : (kp * 2 + 1) * 128, :]
        hi = mat[(kp * 2 + 1) * 128 : (kp * 2 + 2) * 128, :]
        return np.ascontiguousarray(np.concatenate([lo, hi], axis=1))

    in_maps = []
    all_ranks = []
    all_cp = []
    for core in range(NCORES):
        sl = slice(core * NB_SHARD, (core + 1) * NB_SHARD)
        bT = bT8[:, sl]
        c_core = c[core * NB_SHARD : (core + 1) * NB_SHARD]
        ranks = []
        cpair = []
        im = {}
        for kp in range(kp_n):
            im[f"atm0p{kp}"] = pair_pack(aT8[:, : 128 * M0], kp)
            im[f"atrp{kp}"] = pair_pack(aT8[:, 128 * M0 :], kp)
        for g in range(nseg):
            cch = c_core[g * CHUNK : (g + 1) * CHUNK]
            rank = np.argsort(cch, kind="stable")
            ranks.append(rank)
            perm = rank[r_of_q]
            cols = bT[:, g * CHUNK : (g + 1) * CHUNK][:, perm]
            cpair.append(cch[rank.reshape(PAIR, 2)].mean(axis=1).astype(np.float32))
            for kp in range(kp_n):
                full = pair_pack(np.ascontiguousarray(cols), kp)  # [128, 2*CHUNK]
                if g == 0:
                    im[f"bt0p{kp}a"] = np.ascontiguousarray(
                        np.concatenate(
                            [full[:, 0:half], full[:, CHUNK : CHUNK + half]], axis=1
                        )
                    )
                    im[f"bt0p{kp}b"] = np.ascontiguousarray(
                        np.concatenate(
                            [full[:, half:CHUNK], full[:, CHUNK + half :]], axis=1
                        )
                    )
                else:
                    im[f"bt{g}p{kp}"] = full
        in_maps.append(im)
        all_ranks.append(ranks)
        all_cp.append(cpair)
    return in_maps, all_ranks, all_cp


def merge_results(a, b, n, b_batch_size, results, all_ranks, all_cp):
    """Subtract the fp32 pair bias from each chunk's 1024 pair-maxima, keep
    the top-NSEL pairs per (query, chunk), expand each to its 2 c-adjacent
    columns, refine with the exact fp32 reference distance, pick final top-n
    (ties -> lowest index), apply the reference's bookkeeping."""
    nseg = NB_SHARD // CHUNK
    na = a.shape[0]
    cand_parts = []
    for core in range(NCORES):
        # Decode the 2-chunk-packed layout: [(na//256)*128, nseg*2*PAIR] ->
        # [na, nseg, PAIR] with query index q = (mp*2 + off)*128 + r.
        oq = (
            results[core]["out_qp"]
            .astype(np.float32)
            .reshape(na // 256, 128, nseg, 2, PAIR)
            .transpose(0, 3, 1, 2, 4)
            .reshape(na, nseg, PAIR)
        )
        for s in range(nseg):
            adj = oq[:, s, :] - all_cp[core][s][None, :]
            top = np.argpartition(-adj, NSEL, axis=1)[:, :NSEL]
            rank = all_ranks[core][s]
            base = core * NB_SHARD + s * CHUNK
            for k in range(2):
                cand_parts.append(rank[2 * top + k] + base)
    cand = np.concatenate(cand_parts, axis=1)  # [NA, NCORES*nseg*NSEL*2]

    a2 = np.sum(a * a, axis=1)
    sa = np.sum(a, axis=1)
    b2 = np.sum(b * b, axis=1)
    sb = np.sum(b, axis=1)
    d = a.shape[1]
    out = np.empty((na, n), dtype=np.int64)
    CHQ = 128
    eps = np.float32(EPS)
    for q0 in range(0, na, CHQ):
        q1 = min(q0 + CHQ, na)
        Cc = cand[q0:q1]
        Bc = b[Cc]
        cross = np.matmul(Bc, a[q0:q1, :, None])[..., 0].astype(np.float32)
        sq = (
            a2[q0:q1, None]
            + b2[Cc]
            - np.float32(2.0) * cross
            + np.float32(2.0) * eps * (sa[q0:q1, None] - sb[Cc])
            + np.float32(d) * eps * eps
        )
        dist = np.sqrt(np.maximum(sq, np.float32(0.0)))
        ordr = np.lexsort((Cc, dist), axis=1)[:, :n]
        rows = np.arange(q1 - q0)[:, None]
        out[q0:q1] = Cc[rows, ordr]
    buggy = (out % b_batch_size) + (out // b_batch_size)
    return buggy.astype(np.int32)


def kernel(a, b, n, b_batch_size, trace=False):
    from concourse.bass_utils import run_bass_kernel_spmd

    a = np.ascontiguousarray(np.asarray(a, dtype=np.float32))
    b = np.ascontiguousarray(np.asarray(b, dtype=np.float32))
    n = int(n)
    b_batch_size = int(b_batch_size)

    nc = build_kernel()
    in_maps, all_ranks, all_cp = make_in_maps(a, b)
    res = run_bass_kernel_spmd(
        nc, in_maps, core_ids=list(range(NCORES)), trace=trace
    )
    out = merge_results(a, b, n, b_batch_size, res.results, all_ranks, all_cp)
    if trace:
        return out, res
    return out


# revision 31
# speedup vs baseline: 1.0251x; 1.0003x over previous
"""Sharded kNN (ArgDistanceMeasure) on 8 TRN2 NeuronCores.

~129us HW exec at full clock (vs 157us baseline; note the board runs some
executions at ~5/6 clock — warm N=512 matmul 454ns instead of 379ns — which
inflates any measurement by ~20%).  Breakdown: ~7.5us fixed NEFF preamble,
~3.5us warmup/first-DMA, ~111.5us matmul stream (floor: 512 DR matmuls x
216ns = 110.6us), ~2.3us consumer drain, ~2.8us end barrier + sem clears.

Strategy (FAISS-style sharded kNN):
  - b (the database, [65536, 512]) is sharded row-wise across 8 cores
    (8192 rows each); a (queries, [2048, 512]) is replicated.
  - Ranking identity: argmin_j ||a_i - b_j + eps||^2 over j only needs the
    column-dependent part  score[i,j] = 2*a_i.b_j - c_j  (maximized), where
    c_j = ||b_j||^2 - 2*eps*sum(b_j).  Row-constant terms don't affect
    per-row ranking.
  - Columns of each 2048-wide chunk are host-permuted so that device
    position q holds the column with c-sorted rank (q % 1024)*2 + (q // 1024):
    one pairwise-max over the chunk halves groups PAIRS of c-adjacent columns
    (spread <= 2 ranks), so the c_j bias can be applied on the host to the
    1024 pair-maxima — the PE runs a pure GEMM.
  - Per [128 queries x 2048 cols] chunk:
      PE:  fp8-e4m3 DoubleRow GEMM (K=256/matmul) accumulating 2*cross into
           PSUM; 8 matmuls of N=512 (~1.73us/chunk warm).
      ACT: two 512-col copies PSUM[0:1024] -> SBUF fp16 (s16a).
      DVE: two 512-col tensor_max ops, each reading one s16a half (SBUF
           fp16) and one PSUM fp32 quadrant directly (dual-PSUM-operand TT
           is rejected by walrus; one PSUM operand is fine) -> m2 fp16
           pair-maxima.  Each PSUM quadrant has exactly ONE consumer
           (ACT-a, ACT-b, L1a, L1b), so every next-next-chunk matmul
           carries a single semaphore wait, and the quadrants free
           in a staggered pipeline well before the PE needs them.
      DMA: pair-maxima packed fp8, TWO m-chunks per [128, 2048] tile
           (128KB per 2 chunks) -> DRAM out_qp.  fp8 selection noise is
           validated safe (worst true-member rank 5 of the 32 kept); the
           final out-DMA's completion gates the kernel-end barrier, so
           halving output bytes and doubling DMA line size cuts exec tail.
  - Startup: the first m-slice of a (queries 0-511) and the first half of
    b-chunk 0 are small dedicated DMAs so the first matmul issues ~4us
    earlier than a monolithic preload; remaining loads are dependency-gated
    behind the critical wave.  Dummy matmuls on memset scratch warm the PE
    HAM clock (cold 1.2GHz -> warm 2.4GHz) while the first DMAs land.
  - Host: subtracts the fp32 per-pair bias from the 1024 pair-maxima per
    chunk, keeps the top-32 pairs per (query, chunk), expands each to its 2
    c-adjacent columns (2048 candidates/query), recomputes the exact fp32
    reference distance and picks the final top-n with the reference's
    tie-break and buggy index bookkeeping.  (Validated in simulation:
    0/32768 mismatches.)
"""
import numpy as np

NA, D, NB = 2048, 512, 65536
NCORES = 8
NB_SHARD = NB // NCORES  # 8192
CHUNK = 2048             # chunk width (4 PSUM banks)
PAIR = CHUNK // 2        # 1024 pair-maxima per chunk
NSEL = 32                # pairs kept per (query, chunk) on the host
NDUMMY = 7               # PE warmup matmuls (N=512) on scratch: span the
                         # first-DMA wait AND the ~3.4us HAM activity window,
                         # so real matmuls start warm (2.4GHz) as soon as
                         # their data lands
EPS = 1e-6
M0 = 4                   # m-tiles covered by the first a m-slice


def build_kernel(na=NA, nb_shard=NB_SHARD, chunk=CHUNK):
    import concourse.mybir as mybir
    from concourse import bacc
    from concourse.tile import TileContext, add_dep_helper

    FP8 = mybir.dt.float8e4
    F16 = mybir.dt.float16
    F32 = mybir.dt.float32
    DR = mybir.MatmulPerfMode.DoubleRow

    nseg = nb_shard // chunk
    kt = D // 128            # 4 K-tiles of 128
    kp_n = kt // 2           # 2 DoubleRow pairs (K=256 each)
    mt = na // 128

    nc = bacc.Bacc()

    # DoubleRow operands are [128, 2, cols] (two K-slices packed per
    # partition).  a is split into a first m-slice (queries 0..511) and the
    # rest; b chunk 0 is split into halves so the first matmul's operands are
    # small, early DMAs.
    atm0_p = [
        nc.declare_dram_parameter(f"atm0p{kp}", [128, 2 * 128 * M0], FP8, isOutput=False)
        for kp in range(kp_n)
    ]
    atr_p = [
        nc.declare_dram_parameter(
            f"atrp{kp}", [128, 2 * 128 * (mt - M0)], FP8, isOutput=False
        )
        for kp in range(kp_n)
    ]
    bt0h_p = [
        [
            nc.declare_dram_parameter(
                f"bt0p{kp}{h}", [128, 2 * (chunk // 2)], FP8, isOutput=False
            )
            for h in ("a", "b")
        ]
        for kp in range(kp_n)
    ]
    bt_p = {
        (g, kp): nc.declare_dram_parameter(
            f"bt{g}p{kp}", [128, 2 * chunk], FP8, isOutput=False
        )
        for g in range(1, nseg)
        for kp in range(kp_n)
    }
    # Pair-maxima output, fp8 (selection margin validated: worst true-member
    # rank 5 of 32 kept), packed TWO m-chunks per row-block so each out-DMA
    # moves a [128, 2048] tile with contiguous 2KB per-partition lines:
    # row (m//2)*128 + r, col s*2048 + (m%2)*1024 + p  holds the value for
    # (query m*128+r, chunk s, pair p).  Halves output bytes and trigger
    # count vs per-chunk fp16 — the final out-DMA's completion gates the
    # kernel-end barrier, so queue backlog here is exec time.
    out_qp = nc.declare_dram_parameter(
        "out_qp", [(na // 256) * 128, nseg * 2 * PAIR], FP8, isOutput=True
    )

    with TileContext(nc) as tc:
        with (
            tc.tile_pool(name="weights", bufs=1) as wpool,
            tc.tile_pool(name="psum", bufs=2, space="PSUM") as ppool,
            tc.tile_pool(name="scores", bufs=4) as spool,
            tc.tile_pool(name="pairs", bufs=8) as mpool,
        ):
            # --- PE warmup: dummy DoubleRow matmuls on memset scratch ---
            wscr = wpool.tile([128, 2 * 512], FP8, tag="wscr", name="wscr")
            nc.gpsimd.memset(wscr, 0)
            w3 = wscr.rearrange("p (i c) -> p i c", i=2)
            # Per-j PSUM tiles (4 tags x bufs=2 x 1 bank = all 8 banks): each
            # consumer (ACT-a<-j0, ACT-b<-j1, L1a<-j2, L1b<-j3) then depends
            # on just its own quadrant's two matmuls, so copies start ~1us
            # before the chunk's last matmul and the PE never waits on the
            # (tile-granular) PSUM WAR chain.
            ps_first = ppool.tile([128, 512], F32, tag="score0", name="ps00")
            for _ in range(NDUMMY):
                nc.tensor.matmul(
                    ps_first, w3[:, :, :128], w3,
                    start=True, stop=True, perf_mode=DR,
                )

            # --- critical first wave: a m-slice 0, b chunk 0 halves.
            # The chunk-0 halves are separate CONTIGUOUS tiles: a DMA into a
            # strided [128,2,1024] view of one big tile generates 1KB packets
            # and runs ~4x slower than these 2KB-per-partition lines.
            atm0 = []
            bt0h = [[None, None] for _ in range(kp_n)]
            half = chunk // 2
            for kp in range(kp_n):
                for hh in range(2):
                    bt0h[kp][hh] = wpool.tile(
                        [128, 2 * half], FP8, tag=f"bt0p{kp}h{hh}", name=f"bt0p{kp}h{hh}"
                    )
            crit0 = nc.sync.dma_start(out=bt0h[0][0], in_=bt0h_p[0][0][:, :])
            for kp in range(kp_n):
                t = wpool.tile(
                    [128, 2 * 128 * M0], FP8, tag=f"atm0p{kp}", name=f"atm0p{kp}"
                )
                nc.sync.dma_start(out=t, in_=atm0_p[kp][:, :])
                atm0.append(t)
            nc.sync.dma_start(out=bt0h[1][0], in_=bt0h_p[1][0][:, :])
            for kp in range(kp_n):
                crit_dma = nc.sync.dma_start(out=bt0h[kp][1], in_=bt0h_p[kp][1][:, :])
            bt0h3 = [
                [t.rearrange("p (i c) -> p i c", i=2) for t in row] for row in bt0h
            ]
            # --- gated preloads: rest of a, b chunks 1..3 ---
            atr = []
            for kp in range(kp_n):
                t = wpool.tile(
                    [128, 2 * 128 * (mt - M0)], FP8, tag=f"atrp{kp}", name=f"atrp{kp}"
                )
                d = nc.sync.dma_start(out=t, in_=atr_p[kp][:, :])
                add_dep_helper(d.ins, crit_dma.ins, True, "preload priority")
                atr.append(t)
            # bt1 rides with atr behind the critical wave; bt2/bt3 are a
            # SECOND gated wave behind bt1 so the out-DMA packets (whose
            # final completion gates the kernel-end barrier) start draining
            # mid-stream instead of queueing behind 4MB of preloads.
            bt_t = {}
            wave1 = None
            for g in range(1, nseg):
                for kp in range(kp_n):
                    t = wpool.tile(
                        [128, 2 * chunk], FP8, tag=f"bt{g}p{kp}", name=f"bt{g}p{kp}"
                    )
                    gate = crit_dma if g == 1 else wave1
                    d = nc.sync.dma_start(out=t, in_=bt_p[(g, kp)][:, :])
                    add_dep_helper(d.ins, gate.ins, True, "preload priority")
                    if g == 1:
                        wave1 = d
                    bt_t[(g, kp)] = t

            atm0_3 = [t.rearrange("p (i c) -> p i c", i=2) for t in atm0]
            atr_3 = [t.rearrange("p (i c) -> p i c", i=2) for t in atr]
            bt3 = {
                (g, kp): bt_t[(g, kp)].rearrange("p (i c) -> p i c", i=2)
                for g in range(1, nseg)
                for kp in range(kp_n)
            }

            for s in range(nseg):
                for m in range(mt):
                    ps_j = []
                    for j in range(4):
                        if s == 0 and m == 0 and j == 0:
                            ps_j.append(ps_first)
                        else:
                            ps_j.append(
                                ppool.tile(
                                    [128, 512], F32, tag=f"score{j}", name=f"ps{j}"
                                )
                            )
                    # j grouped in halves so chunk 0's matmuls chase the
                    # half-DMAs; kp inner per half for PSUM accumulate.
                    for h in range(2):
                        for kp in range(kp_n):
                            if m < M0:
                                lhsT3 = atm0_3[kp][:, :, m * 128 : (m + 1) * 128]
                            else:
                                lhsT3 = atr_3[kp][:, :, (m - M0) * 128 : (m - M0 + 1) * 128]
                            for j in (2 * h, 2 * h + 1):
                                if s == 0:
                                    rhs3 = bt0h3[kp][h][:, :, (j % 2) * 512 : (j % 2 + 1) * 512]
                                else:
                                    rhs3 = bt3[(s, kp)][:, :, j * 512 : (j + 1) * 512]
                                nc.tensor.matmul(
                                    ps_j[j],
                                    lhsT3,
                                    rhs3,
                                    start=(kp == 0),
                                    stop=(kp == kp_n - 1),
                                    perf_mode=DR,
                                )
                    s16a = spool.tile([128, PAIR], F16, tag="s16a", name="s16a")
                    nc.scalar.copy(out=s16a[:, :512], in_=ps_j[0])
                    nc.scalar.copy(out=s16a[:, 512:], in_=ps_j[1])
                    if m % 2 == 0:
                        m2 = mpool.tile([128, 2 * PAIR], FP8, tag="m2", name="m2")
                    off = (m % 2) * PAIR
                    nc.vector.tensor_max(
                        m2[:, off : off + 512], s16a[:, :512], ps_j[2]
                    )
                    nc.vector.tensor_max(
                        m2[:, off + 512 : off + PAIR], s16a[:, 512:], ps_j[3]
                    )
                    last_pair = s == nseg - 1 and m >= mt - 2
                    if last_pair:
                        # Final pair: one half-DMA per chunk so the kernel-end
                        # barrier only waits on a 128KB transfer.
                        mp_ = m // 2
                        nc.sync.dma_start(
                            out=out_qp[
                                mp_ * 128 : (mp_ + 1) * 128,
                                s * 2 * PAIR + off : s * 2 * PAIR + off + PAIR,
                            ],
                            in_=m2[:, off : off + PAIR],
                        )
                    elif m % 2 == 1:
                        mp_ = m // 2
                        nc.sync.dma_start(
                            out=out_qp[
                                mp_ * 128 : (mp_ + 1) * 128,
                                s * 2 * PAIR : (s + 1) * 2 * PAIR,
                            ],
                            in_=m2,
                        )
    nc.compile()
    return nc


def make_in_maps(a, b):
    """Pack per-core inputs.  Columns of each 2048-wide chunk are permuted so
    that device position q holds the column with c-sorted rank
    (q % 1024)*2 + (q // 1024) — making the pairwise-max mates c-adjacent
    (spread <= 2 ranks), so the bias can be applied on the host to the 1024
    pair-maxima.  Returns (in_maps, ranks, cpair) where ranks[core][s][r] is
    the local column with the r-th smallest c and cpair[core][s][t] the mean
    c of pair t."""
    import ml_dtypes

    kt = D // 128
    kp_n = kt // 2
    aT8 = (2.0 * a).T.astype(ml_dtypes.float8_e4m3)   # [512, NA]
    bT8 = b.T.astype(ml_dtypes.float8_e4m3)           # [512, NB]
    b2 = np.sum(b * b, axis=1)
    sb = b.sum(axis=1)
    c = (b2 - np.float32(2.0 * EPS) * sb).astype(np.float32)
    nseg = NB_SHARD // CHUNK
    q = np.arange(CHUNK)
    r_of_q = (q % PAIR) * 2 + (q // PAIR)
    half = CHUNK // 2

    def pair_pack(mat, kp):
        # [128, 2*cols]: slot i holds K-tile (kp*2+i) rows of mat
        lo = mat[(kp * 2) * 128 : (kp * 2 + 1) * 128, :]
        hi = mat[(kp * 2 + 1) * 128 : (kp * 2 + 2) * 128, :]
        return np.ascontiguousarray(np.concatenate([lo, hi], axis=1))

    in_maps = []
    all_ranks = []
    all_cp = []
    for core in range(NCORES):
        sl = slice(core * NB_SHARD, (core + 1) * NB_SHARD)
        bT = bT8[:, sl]
        c_core = c[core * NB_SHARD : (core + 1) * NB_SHARD]
        ranks = []
        cpair = []
        im = {}
        for kp in range(kp_n):
            im[f"atm0p{kp}"] = pair_pack(aT8[:, : 128 * M0], kp)
            im[f"atrp{kp}"] = pair_pack(aT8[:, 128 * M0 :], kp)
        for g in range(nseg):
            cch = c_core[g * CHUNK : (g + 1) * CHUNK]
            rank = np.argsort(cch, kind="stable")
            ranks.append(rank)
            perm = rank[r_of_q]
            cols = bT[:, g * CHUNK : (g + 1) * CHUNK][:, perm]
            cpair.append(cch[rank.reshape(PAIR, 2)].mean(axis=1).astype(np.float32))
            for kp in range(kp_n):
                full = pair_pack(np.ascontiguousarray(cols), kp)  # [128, 2*CHUNK]
                if g == 0:
                    im[f"bt0p{kp}a"] = np.ascontiguousarray(
                        np.concatenate(
                            [full[:, 0:half], full[:, CHUNK : CHUNK + half]], axis=1
                        )
                    )
                    im[f"bt0p{kp}b"] = np.ascontiguousarray(
                        np.concatenate(
                            [full[:, half:CHUNK], full[:, CHUNK + half :]], axis=1
                        )
                    )
                else:
                    im[f"bt{g}p{kp}"] = full
        in_maps.append(im)
        all_ranks.append(ranks)
        all_cp.append(cpair)
    return in_maps, all_ranks, all_cp


def merge_results(a, b, n, b_batch_size, results, all_ranks, all_cp):
    """Subtract the fp32 pair bias from each chunk's 1024 pair-maxima, keep
    the top-NSEL pairs per (query, chunk), expand each to its 2 c-adjacent
    columns, refine with the exact fp32 reference distance, pick final top-n
    (ties -> lowest index), apply the reference's bookkeeping."""
    nseg = NB_SHARD // CHUNK
    na = a.shape[0]
    cand_parts = []
    for core in range(NCORES):
        # Decode the 2-chunk-packed layout: [(na//256)*128, nseg*2*PAIR] ->
        # [na, nseg, PAIR] with query index q = (mp*2 + off)*128 + r.
        oq = (
            results[core]["out_qp"]
            .astype(np.float32)
            .reshape(na // 256, 128, nseg, 2, PAIR)
            .transpose(0, 3, 1, 2, 4)
            .reshape(na, nseg, PAIR)
        )
        for s in range(nseg):
            adj = oq[:, s, :] - all_cp[core][s][None, :]
            top = np.argpartition(-adj, NSEL, axis=1)[:, :NSEL]
            rank = all_ranks[core][s]
            base = core * NB_SHARD + s * CHUNK
            for k in range(2):
                cand_parts.append(rank[2 * top + k] + base)
    cand = np.concatenate(cand_parts, axis=1)  # [NA, NCORES*nseg*NSEL*2]

    a2 = np.sum(a * a, axis=1)
    sa = np.sum(a, axis=1)
    b2 = np.sum(b * b, axis=1)
    sb = np.sum(b, axis=1)
    d = a.shape[1]
    out = np.empty((na, n), dtype=np.int64)
    CHQ = 128
    eps = np.float32(EPS)
    for q0 in range(0, na, CHQ):
        q1 = min(q0 + CHQ, na)
        Cc = cand[q0:q1]
        Bc = b[Cc]
        cross = np.matmul(Bc, a[q0:q1, :, None])[..., 0].astype(np.float32)
        sq = (
            a2[q0:q1, None]
            + b2[Cc]
            - np.float32(2.0) * cross
            + np.float32(2.0) * eps * (sa[q0:q1, None] - sb[Cc])
            + np.float32(d) * eps * eps
        )
        dist = np.sqrt(np.maximum(sq, np.float32(0.0)))
        ordr = np.lexsort((Cc, dist), axis=1)[:, :n]
        rows = np.arange(q1 - q0)[:, None]
        out[q0:q1] = Cc[rows, ordr]
    buggy = (out % b_batch_size) + (out // b_batch_size)
    return buggy.astype(np.int32)


def kernel(a, b, n, b_batch_size, trace=False):
    from concourse.bass_utils import run_bass_kernel_spmd

    a = np.ascontiguousarray(np.asarray(a, dtype=np.float32))
    b = np.ascontiguousarray(np.asarray(b, dtype=np.float32))
    n = int(n)
    b_batch_size = int(b_batch_size)

    nc = build_kernel()
    in_maps, all_ranks, all_cp = make_in_maps(a, b)
    res = run_bass_kernel_spmd(
        nc, in_maps, core_ids=list(range(NCORES)), trace=trace
    )
    out = merge_results(a, b, n, b_batch_size, res.results, all_ranks, all_cp)
    if trace:
        return out, res
    return out


# revision 35
# speedup vs baseline: 1.0275x; 1.0024x over previous
"""Sharded kNN (ArgDistanceMeasure) on 8 TRN2 NeuronCores.

~129us HW exec at full clock (vs 157us baseline; note the board runs some
executions at ~5/6 clock — warm N=512 matmul 454ns instead of 379ns — which
inflates any measurement by ~20%).  Breakdown: ~7.5us fixed NEFF preamble,
~3.5us warmup/first-DMA, ~111.5us matmul stream (floor: 512 DR matmuls x
216ns = 110.6us), ~2.3us consumer drain, ~2.8us end barrier + sem clears.

Strategy (FAISS-style sharded kNN):
  - b (the database, [65536, 512]) is sharded row-wise across 8 cores
    (8192 rows each); a (queries, [2048, 512]) is replicated.
  - Ranking identity: argmin_j ||a_i - b_j + eps||^2 over j only needs the
    column-dependent part  score[i,j] = 2*a_i.b_j - c_j  (maximized), where
    c_j = ||b_j||^2 - 2*eps*sum(b_j).  Row-constant terms don't affect
    per-row ranking.
  - Columns of each 2048-wide chunk are host-permuted so that device
    position q holds the column with c-sorted rank (q % 1024)*2 + (q // 1024):
    one pairwise-max over the chunk halves groups PAIRS of c-adjacent columns
    (spread <= 2 ranks), so the c_j bias can be applied on the host to the
    1024 pair-maxima — the PE runs a pure GEMM.
  - Per [128 queries x 2048 cols] chunk:
      PE:  fp8-e4m3 DoubleRow GEMM (K=256/matmul) accumulating 2*cross into
           PSUM; 8 matmuls of N=512 (~1.73us/chunk warm).
      ACT: two 512-col copies PSUM[0:1024] -> SBUF fp16 (s16a).
      DVE: two 512-col tensor_max ops, each reading one s16a half (SBUF
           fp16) and one PSUM fp32 quadrant directly (dual-PSUM-operand TT
           is rejected by walrus; one PSUM operand is fine) -> m2 fp16
           pair-maxima.  Each PSUM quadrant has exactly ONE consumer
           (ACT-a, ACT-b, L1a, L1b), so every next-next-chunk matmul
           carries a single semaphore wait, and the quadrants free
           in a staggered pipeline well before the PE needs them.
      DMA: pair-maxima packed fp8, TWO m-chunks per [128, 2048] tile
           (128KB per 2 chunks) -> DRAM out_qp.  fp8 selection noise is
           validated safe (worst true-member rank 5 of the 32 kept); the
           final out-DMA's completion gates the kernel-end barrier, so
           halving output bytes and doubling DMA line size cuts exec tail.
  - Startup: the first m-slice of a (queries 0-511) and the first half of
    b-chunk 0 are small dedicated DMAs so the first matmul issues ~4us
    earlier than a monolithic preload; remaining loads are dependency-gated
    behind the critical wave.  Dummy matmuls on memset scratch warm the PE
    HAM clock (cold 1.2GHz -> warm 2.4GHz) while the first DMAs land.
  - Host: subtracts the fp32 per-pair bias from the 1024 pair-maxima per
    chunk, keeps the top-32 pairs per (query, chunk), expands each to its 2
    c-adjacent columns (2048 candidates/query), recomputes the exact fp32
    reference distance and picks the final top-n with the reference's
    tie-break and buggy index bookkeeping.  (Validated in simulation:
    0/32768 mismatches.)
"""
import numpy as np

NA, D, NB = 2048, 512, 65536
NCORES = 8
NB_SHARD = NB // NCORES  # 8192
CHUNK = 2048             # chunk width (4 PSUM banks)
PAIR = CHUNK // 2        # 1024 pair-maxima per chunk
NSEL = 32                # pairs kept per (query, chunk) on the host
NDUMMY = 7               # PE warmup matmuls (N=512) on scratch: span the
                         # first-DMA wait AND the ~3.4us HAM activity window,
                         # so real matmuls start warm (2.4GHz) as soon as
                         # their data lands
EPS = 1e-6
M0 = 4                   # m-tiles covered by the first a m-slice


def build_kernel(na=NA, nb_shard=NB_SHARD, chunk=CHUNK):
    import concourse.mybir as mybir
    from concourse import bacc
    from concourse.tile import TileContext, add_dep_helper

    FP8 = mybir.dt.float8e4
    F16 = mybir.dt.float16
    F32 = mybir.dt.float32
    DR = mybir.MatmulPerfMode.DoubleRow

    nseg = nb_shard // chunk
    kt = D // 128            # 4 K-tiles of 128
    kp_n = kt // 2           # 2 DoubleRow pairs (K=256 each)
    mt = na // 128

    nc = bacc.Bacc()

    # DoubleRow operands are [128, 2, cols] (two K-slices packed per
    # partition).  a is split into a first m-slice (queries 0..511) and the
    # rest; b chunk 0 is split into halves so the first matmul's operands are
    # small, early DMAs.
    # Both kp-pairs of the first a m-slice in ONE param: 2KB-per-partition
    # DMA lines (vs 1KB split per kp) and a single trigger.
    atm0_p = nc.declare_dram_parameter(
        "atm0", [128, 2 * 2 * 128 * M0], FP8, isOutput=False
    )
    atr_p = [
        nc.declare_dram_parameter(
            f"atrp{kp}", [128, 2 * 128 * (mt - M0)], FP8, isOutput=False
        )
        for kp in range(kp_n)
    ]
    bt0h_p = [
        [
            nc.declare_dram_parameter(
                f"bt0p{kp}{h}", [128, 2 * (chunk // 2)], FP8, isOutput=False
            )
            for h in ("a", "b")
        ]
        for kp in range(kp_n)
    ]
    bt_p = {
        (g, kp): nc.declare_dram_parameter(
            f"bt{g}p{kp}", [128, 2 * chunk], FP8, isOutput=False
        )
        for g in range(1, nseg)
        for kp in range(kp_n)
    }
    # Pair-maxima output, fp8 (selection margin validated: worst true-member
    # rank 5 of 32 kept), packed TWO m-chunks per row-block so each out-DMA
    # moves a [128, 2048] tile with contiguous 2KB per-partition lines:
    # row (m//2)*128 + r, col s*2048 + (m%2)*1024 + p  holds the value for
    # (query m*128+r, chunk s, pair p).  Halves output bytes and trigger
    # count vs per-chunk fp16 — the final out-DMA's completion gates the
    # kernel-end barrier, so queue backlog here is exec time.
    out_qp = nc.declare_dram_parameter(
        "out_qp", [(na // 256) * 128, nseg * 2 * PAIR], FP8, isOutput=True
    )

    with TileContext(nc) as tc:
        with (
            tc.tile_pool(name="weights", bufs=1) as wpool,
            tc.tile_pool(name="psum", bufs=2, space="PSUM") as ppool,
            tc.tile_pool(name="scores", bufs=4) as spool,
            tc.tile_pool(name="pairs", bufs=8) as mpool,
        ):
            # --- PE warmup: dummy DoubleRow matmuls on memset scratch ---
            # memset on the Vector engine: it clears its preamble ~1.4us
            # before GpSimd, so the warmup matmuls (and hence the HAM
            # warm-clock flip) start earlier.
            wscr = wpool.tile([128, 2 * 512], FP8, tag="wscr", name="wscr")
            nc.vector.memset(wscr, 0)
            w3 = wscr.rearrange("p (i c) -> p i c", i=2)
            # Per-j PSUM tiles (4 tags x bufs=2 x 1 bank = all 8 banks): each
            # consumer (ACT-a<-j0, ACT-b<-j1, L1a<-j2, L1b<-j3) then depends
            # on just its own quadrant's two matmuls, so copies start ~1us
            # before the chunk's last matmul and the PE never waits on the
            # (tile-granular) PSUM WAR chain.
            ps_first = ppool.tile([128, 512], F32, tag="score0", name="ps00")
            for _ in range(NDUMMY):
                nc.tensor.matmul(
                    ps_first, w3[:, :, :128], w3,
                    start=True, stop=True, perf_mode=DR,
                )

            # --- critical first wave: a m-slice 0, b chunk 0 halves.
            # The chunk-0 halves are separate CONTIGUOUS tiles: a DMA into a
            # strided [128,2,1024] view of one big tile generates 1KB packets
            # and runs ~4x slower than these 2KB-per-partition lines.
            atm0 = []
            bt0h = [[None, None] for _ in range(kp_n)]
            half = chunk // 2
            for kp in range(kp_n):
                for hh in range(2):
                    bt0h[kp][hh] = wpool.tile(
                        [128, 2 * half], FP8, tag=f"bt0p{kp}h{hh}", name=f"bt0p{kp}h{hh}"
                    )
            crit0 = nc.sync.dma_start(out=bt0h[0][0], in_=bt0h_p[0][0][:, :])
            atm0t = wpool.tile(
                [128, 2 * 2 * 128 * M0], FP8, tag="atm0", name="atm0t"
            )
            nc.sync.dma_start(out=atm0t, in_=atm0_p[:, :])
            atm0 = [atm0t[:, kp * 2 * 128 * M0 : (kp + 1) * 2 * 128 * M0]
                    for kp in range(kp_n)]
            nc.sync.dma_start(out=bt0h[1][0], in_=bt0h_p[1][0][:, :])
            for kp in range(kp_n):
                crit_dma = nc.sync.dma_start(out=bt0h[kp][1], in_=bt0h_p[kp][1][:, :])
            bt0h3 = [
                [t.rearrange("p (i c) -> p i c", i=2) for t in row] for row in bt0h
            ]
            # --- gated preloads: rest of a, b chunks 1..3 ---
            atr = []
            for kp in range(kp_n):
                t = wpool.tile(
                    [128, 2 * 128 * (mt - M0)], FP8, tag=f"atrp{kp}", name=f"atrp{kp}"
                )
                d = nc.sync.dma_start(out=t, in_=atr_p[kp][:, :])
                add_dep_helper(d.ins, crit_dma.ins, True, "preload priority")
                atr.append(t)
            # bt1 rides with atr behind the critical wave; bt2/bt3 are a
            # SECOND gated wave behind bt1 so the out-DMA packets (whose
            # final completion gates the kernel-end barrier) start draining
            # mid-stream instead of queueing behind 4MB of preloads.
            bt_t = {}
            wave1 = None
            for g in range(1, nseg):
                for kp in range(kp_n):
                    t = wpool.tile(
                        [128, 2 * chunk], FP8, tag=f"bt{g}p{kp}", name=f"bt{g}p{kp}"
                    )
                    gate = crit_dma if g == 1 else wave1
                    d = nc.sync.dma_start(out=t, in_=bt_p[(g, kp)][:, :])
                    add_dep_helper(d.ins, gate.ins, True, "preload priority")
                    if g == 1:
                        wave1 = d
                    bt_t[(g, kp)] = t

            atm0_3 = [t.rearrange("p (i c) -> p i c", i=2) for t in atm0]
            atr_3 = [t.rearrange("p (i c) -> p i c", i=2) for t in atr]
            bt3 = {
                (g, kp): bt_t[(g, kp)].rearrange("p (i c) -> p i c", i=2)
                for g in range(1, nseg)
                for kp in range(kp_n)
            }

            for s in range(nseg):
                for m in range(mt):
                    ps_j = []
                    for j in range(4):
                        if s == 0 and m == 0 and j == 0:
                            ps_j.append(ps_first)
                        else:
                            ps_j.append(
                                ppool.tile(
                                    [128, 512], F32, tag=f"score{j}", name=f"ps{j}"
                                )
                            )
                    # j grouped in halves so chunk 0's matmuls chase the
                    # half-DMAs; kp inner per half for PSUM accumulate.
                    for h in range(2):
                        for kp in range(kp_n):
                            if m < M0:
                                lhsT3 = atm0_3[kp][:, :, m * 128 : (m + 1) * 128]
                            else:
                                lhsT3 = atr_3[kp][:, :, (m - M0) * 128 : (m - M0 + 1) * 128]
                            for j in (2 * h, 2 * h + 1):
                                if s == 0:
                                    rhs3 = bt0h3[kp][h][:, :, (j % 2) * 512 : (j % 2 + 1) * 512]
                                else:
                                    rhs3 = bt3[(s, kp)][:, :, j * 512 : (j + 1) * 512]
                                nc.tensor.matmul(
                                    ps_j[j],
                                    lhsT3,
                                    rhs3,
                                    start=(kp == 0),
                                    stop=(kp == kp_n - 1),
                                    perf_mode=DR,
                                )
                    s16a = spool.tile([128, PAIR], F16, tag="s16a", name="s16a")
                    nc.scalar.copy(out=s16a[:, :512], in_=ps_j[0])
                    nc.scalar.copy(out=s16a[:, 512:], in_=ps_j[1])
                    if m % 2 == 0:
                        m2 = mpool.tile([128, 2 * PAIR], FP8, tag="m2", name="m2")
                    off = (m % 2) * PAIR
                    nc.vector.tensor_max(
                        m2[:, off : off + 512], s16a[:, :512], ps_j[2]
                    )
                    nc.vector.tensor_max(
                        m2[:, off + 512 : off + PAIR], s16a[:, 512:], ps_j[3]
                    )
                    last_pair = s == nseg - 1 and m >= mt - 2
                    if last_pair:
                        # Final pair: one half-DMA per chunk so the kernel-end
                        # barrier only waits on a 128KB transfer.
                        mp_ = m // 2
                        nc.sync.dma_start(
                            out=out_qp[
                                mp_ * 128 : (mp_ + 1) * 128,
                                s * 2 * PAIR + off : s * 2 * PAIR + off + PAIR,
                            ],
                            in_=m2[:, off : off + PAIR],
                        )
                    elif m % 2 == 1:
                        mp_ = m // 2
                        nc.sync.dma_start(
                            out=out_qp[
                                mp_ * 128 : (mp_ + 1) * 128,
                                s * 2 * PAIR : (s + 1) * 2 * PAIR,
                            ],
                            in_=m2,
                        )
    nc.compile()
    return nc


def make_in_maps(a, b):
    """Pack per-core inputs.  Columns of each 2048-wide chunk are permuted so
    that device position q holds the column with c-sorted rank
    (q % 1024)*2 + (q // 1024) — making the pairwise-max mates c-adjacent
    (spread <= 2 ranks), so the bias can be applied on the host to the 1024
    pair-maxima.  Returns (in_maps, ranks, cpair) where ranks[core][s][r] is
    the local column with the r-th smallest c and cpair[core][s][t] the mean
    c of pair t."""
    import ml_dtypes

    kt = D // 128
    kp_n = kt // 2
    aT8 = (2.0 * a).T.astype(ml_dtypes.float8_e4m3)   # [512, NA]
    bT8 = b.T.astype(ml_dtypes.float8_e4m3)           # [512, NB]
    b2 = np.sum(b * b, axis=1)
    sb = b.sum(axis=1)
    c = (b2 - np.float32(2.0 * EPS) * sb).astype(np.float32)
    nseg = NB_SHARD // CHUNK
    q = np.arange(CHUNK)
    r_of_q = (q % PAIR) * 2 + (q // PAIR)
    half = CHUNK // 2

    def pair_pack(mat, kp):
        # [128, 2*cols]: slot i holds K-tile (kp*2+i) rows of mat
        lo = mat[(kp * 2) * 128 : (kp * 2 + 1) * 128, :]
        hi = mat[(kp * 2 + 1) * 128 : (kp * 2 + 2) * 128, :]
        return np.ascontiguousarray(np.concatenate([lo, hi], axis=1))

    in_maps = []
    all_ranks = []
    all_cp = []
    for core in range(NCORES):
        sl = slice(core * NB_SHARD, (core + 1) * NB_SHARD)
        bT = bT8[:, sl]
        c_core = c[core * NB_SHARD : (core + 1) * NB_SHARD]
        ranks = []
        cpair = []
        im = {}
        im["atm0"] = np.ascontiguousarray(
            np.concatenate(
                [pair_pack(aT8[:, : 128 * M0], kp) for kp in range(kp_n)], axis=1
            )
        )
        for kp in range(kp_n):
            im[f"atrp{kp}"] = pair_pack(aT8[:, 128 * M0 :], kp)
        for g in range(nseg):
            cch = c_core[g * CHUNK : (g + 1) * CHUNK]
            rank = np.argsort(cch, kind="stable")
            ranks.append(rank)
            perm = rank[r_of_q]
            cols = bT[:, g * CHUNK : (g + 1) * CHUNK][:, perm]
            cpair.append(cch[rank.reshape(PAIR, 2)].mean(axis=1).astype(np.float32))
            for kp in range(kp_n):
                full = pair_pack(np.ascontiguousarray(cols), kp)  # [128, 2*CHUNK]
                if g == 0:
                    im[f"bt0p{kp}a"] = np.ascontiguousarray(
                        np.concatenate(
                            [full[:, 0:half], full[:, CHUNK : CHUNK + half]], axis=1
                        )
                    )
                    im[f"bt0p{kp}b"] = np.ascontiguousarray(
                        np.concatenate(
                            [full[:, half:CHUNK], full[:, CHUNK + half :]], axis=1
                        )
                    )
                else:
                    im[f"bt{g}p{kp}"] = full
        in_maps.append(im)
        all_ranks.append(ranks)
        all_cp.append(cpair)
    return in_maps, all_ranks, all_cp


def merge_results(a, b, n, b_batch_size, results, all_ranks, all_cp):
    """Subtract the fp32 pair bias from each chunk's 1024 pair-maxima, keep
    the top-NSEL pairs per (query, chunk), expand each to its 2 c-adjacent
    columns, refine with the exact fp32 reference distance, pick final top-n
    (ties -> lowest index), apply the reference's bookkeeping."""
    nseg = NB_SHARD // CHUNK
    na = a.shape[0]
    cand_parts = []
    for core in range(NCORES):
        # Decode the 2-chunk-packed layout: [(na//256)*128, nseg*2*PAIR] ->
        # [na, nseg, PAIR] with query index q = (mp*2 + off)*128 + r.
        oq = (
            results[core]["out_qp"]
            .astype(np.float32)
            .reshape(na // 256, 128, nseg, 2, PAIR)
            .transpose(0, 3, 1, 2, 4)
            .reshape(na, nseg, PAIR)
        )
        for s in range(nseg):
            adj = oq[:, s, :] - all_cp[core][s][None, :]
            top = np.argpartition(-adj, NSEL, axis=1)[:, :NSEL]
            rank = all_ranks[core][s]
            base = core * NB_SHARD + s * CHUNK
            for k in range(2):
                cand_parts.append(rank[2 * top + k] + base)
    cand = np.concatenate(cand_parts, axis=1)  # [NA, NCORES*nseg*NSEL*2]

    a2 = np.sum(a * a, axis=1)
    sa = np.sum(a, axis=1)
    b2 = np.sum(b * b, axis=1)
    sb = np.sum(b, axis=1)
    d = a.shape[1]
    out = np.empty((na, n), dtype=np.int64)
    CHQ = 128
    eps = np.float32(EPS)
    for q0 in range(0, na, CHQ):
        q1 = min(q0 + CHQ, na)
        Cc = cand[q0:q1]
        Bc = b[Cc]
        cross = np.matmul(Bc, a[q0:q1, :, None])[..., 0].astype(np.float32)
        sq = (
            a2[q0:q1, None]
            + b2[Cc]
            - np.float32(2.0) * cross
            + np.float32(2.0) * eps * (sa[q0:q1, None] - sb[Cc])
            + np.float32(d) * eps * eps
        )
        dist = np.sqrt(np.maximum(sq, np.float32(0.0)))
        ordr = np.lexsort((Cc, dist), axis=1)[:, :n]
        rows = np.arange(q1 - q0)[:, None]
        out[q0:q1] = Cc[rows, ordr]
    buggy = (out % b_batch_size) + (out // b_batch_size)
    return buggy.astype(np.int32)


def kernel(a, b, n, b_batch_size, trace=False):
    from concourse.bass_utils import run_bass_kernel_spmd

    a = np.ascontiguousarray(np.asarray(a, dtype=np.float32))
    b = np.ascontiguousarray(np.asarray(b, dtype=np.float32))
    n = int(n)
    b_batch_size = int(b_batch_size)

    nc = build_kernel()
    in_maps, all_ranks, all_cp = make_in_maps(a, b)
    res = run_bass_kernel_spmd(
        nc, in_maps, core_ids=list(range(NCORES)), trace=trace
    )
    out = merge_results(a, b, n, b_batch_size, res.results, all_ranks, all_cp)
    if trace:
        return out, res
    return out
